# revision 2
# baseline (speedup 1.0000x reference)
"""LSTM layer kernel for Trainium2 (8 NeuronCores, Bass/Tile).

Reference computation (fp32):
    z = concat([x, h], axis=1)                 # [B, IN+OUT]
    f = sigmoid(z @ w_f + b_f)
    i = sigmoid(z @ w_i + b_i)
    g = tanh   (z @ w_c + b_c)
    o = sigmoid(z @ w_o + b_o)
    c_new = c * f + i * g
    h_new = tanh(c_new) * o                    # [B, OUT]

Shapes: B=4096, IN=OUT=1024, K=IN+OUT=2048.
Sharding (8 cores): 4 batch-groups x 2 output-column-groups; core (i, j)
computes h_new[i*1024:(i+1)*1024, j*512:(j+1)*512].  No collectives.

The PE is the sole bottleneck (ACT/DVE sit at ~13% duty), so the win over
the all-bf16 schedule is cutting PE cycles with fp8 DoubleRow matmuls:
e4m3 operands + perf_mode=DoubleRow contract 256 k-elements per 512-cycle
matmul (2 fp8 products per PE cell per cycle) — 2x bf16 FLOP rate.  Pure
e4m3 for all four gates fails the 2e-2 gate (measured 3.1e-2 end to end:
~2.4% RMS quantization noise per operand -> 3.8e-2 pre-activation noise),
but the gates' error sensitivities are skewed (candidate tanh 2.4e-2,
output 1.5e-2, forget 1.1e-2, input 0.8e-2 when quantized alone), so the
forget/input gates run in fp8 (8 DoubleRow matmuls each) while the
candidate/output gates stay bf16 (16 matmuls each): 48/64 of the bf16
slot count, sim rel err 1.37e-2 (hw-validated DoubleRow adds only ~1e-4
internal rounding noise).

fp8 scales are fixed powers of two (z*32, w*1024; e4m3 relative error is
scale-invariant, only clipping matters and |z|<7.5sigma, |w|<2.3x absmax
stay in +-240), undone by the activation's scale operand (2^-15) at zero
cost.  Quantization runs host-side from the f32 inputs (same class of
prep as the baseline's bf16 casts/transposes).

Device layout: contraction dim (k) and output-channel dim (o) sit on SBUF
partitions; zT [2048,1024] bf16 + zQ e4m3 are the moving operands, the
gate-fused weights (wB bf16 for c/o, wQ e4m3 for f/i) are stationary,
out = w.T @ zT lands in PSUM as [o, b] so bias/sigmoid/tanh run straight
out of PSUM.  wQ is laid out gate-outer ([kp, g, ko, p]) so the DoubleRow
lhsT slice [:, g, ko:ko+2, :] has the pair dim contiguous — the exact AP
shape validated on hw.  c rides bf16 (error contribution ~3e-3 rel,
negligible vs the fp8 noise) to keep SBUF at ~179KB/partition.

Scheduling (inherited from the bf16 baseline, measured there):
  - Tile's For_i puts an all-engine barrier on every back edge; the body
    holds NCOPIES=8 problem copies = 32 "oc units" so the ~2.2us barrier
    amortizes /8, and DMAs issued late in iteration n prefetch what n+1
    starts with (slots persist across the barrier).
  - Unit u's compute is preceded by the DMA for unit u+2's weights/c and
    one quarter of the next copy's z (both dtypes), flattening the DMA
    duty cycle (co-running DMA measurably slows the matmul stream).
  - Within a unit the matmul order is ko-outer/gate-inner: the PSUM bank
    rotates on every matmul (all 8 banks accumulate concurrently),
    overlapping each matmul's drain with the next one's fill.  The f/i
    DoubleRow matmuls ride the even ko rounds and stop at ko=14, so
    their ACT drains overlap the last c/o matmul rounds.
  - z/w loads ride the sync HWDGE ring; c/bias loads and h stores ride
    the scalar ring, so a store waiting on compute never head-of-line
    blocks a load.
  - Slot cycling is static: z/zQ 2 slots (copy parity), w 4 slots, c 4
    slots; unit u reads slot u%4 and the prefetch writes slot (u+2)%4,
    consistent across the loop wrap since 32%4==0.
"""

import numpy as np
import ml_dtypes

import concourse.bass as bass
import concourse.tile as tile
from concourse import bacc
from concourse import mybir
from concourse.bass_utils import run_bass_kernel_spmd

P = 128
B_FULL, IN, OUT = 4096, 1024, 1024
K = IN + OUT                 # 2048 contraction
RB, RO = 4, 2                # batch-shards x out-col-shards = 8 cores
B_L = B_FULL // RB           # 1024 batch rows per core
O_L = OUT // RO              # 512 out cols per core
KO = K // P                  # 16 k-chunks
OC = O_L // P                # 4 out chunks per core
NG = 4                       # gates
NT = 512                     # moving free dim per matmul (one PSUM bank)
NB = B_L // NT               # 2 batch tiles
NCOPIES = 8                  # kernel copies per hardware-loop iteration
ZS = 2                       # z slots (copy parity)
NU = NCOPIES * OC            # oc-units per body
WS = 4                       # w slots (divides NU)
CS = 4                       # c slots (divides NU)

SZ = 32.0                    # fp8 scale for z (power of 2: exact)
SW = 1024.0                  # fp8 scale for w_f/w_i
QSCALE = 1.0 / (SZ * SW)     # 2^-15, folded into the activation

F32 = mybir.dt.float32
BF16 = mybir.dt.bfloat16
FP8 = mybir.dt.float8e4
NP_BF16 = ml_dtypes.bfloat16
NP_FP8 = ml_dtypes.float8_e4m3   # TRN FP8_EXP4 bit pattern (max +-240)
GATES = ("f", "i", "c", "o")
FP8_GATES = ("f", "i")       # low-sensitivity gates -> e4m3 DoubleRow
BF_GATES = ("c", "o")        # high-sensitivity gates -> bf16

last_exec_time_ns = None

_NC_CACHE = {}


def _build_nc(loop_r=None, ko_limit=None):
    nc = bacc.Bacc()

    zT = nc.dram_tensor("zT", [K, B_L], BF16, kind="ExternalInput")
    zQ = nc.dram_tensor("zQ", [K, B_L], FP8, kind="ExternalInput")
    cT = nc.dram_tensor("cT", [O_L, B_L], BF16, kind="ExternalInput")
    # bf16 gate weights (c, o): [k, oc, gate, p] with o_local = oc*128 + p
    wB = nc.dram_tensor("wB", [K, OC, 2, P], BF16, kind="ExternalInput")
    # fp8 gate weights (f, i), same dram layout; SBUF goes gate-outer
    wQ = nc.dram_tensor("wQ", [K, OC, 2, P], FP8, kind="ExternalInput")
    # gate-fused biases: [p, oc, gate] in GATES order
    bA = nc.dram_tensor("bA", [P, OC, NG], F32, kind="ExternalInput")
    hT = nc.dram_tensor("hT", [O_L, B_L], BF16, kind="ExternalOutput")

    zT_t = zT[:, :].rearrange("(ko kp) b -> kp ko b", kp=P)    # [128,16,1024]
    zQ_t = zQ[:, :].rearrange("(ko kp) b -> kp ko b", kp=P)
    cT_t = cT[:, :].rearrange("(oc p) b -> p oc b", p=P)       # [128,4,1024]
    hT_t = hT[:, :].rearrange("(oc p) b -> p oc b", p=P)
    wB_t = wB[:, :, :, :].rearrange(
        "(ko kp) oc g p -> kp ko oc (g p)", kp=P
    )                                                          # [128,16,4,256]
    wQ_t = wQ[:, :, :, :].rearrange(
        "(ko kp) oc g p -> kp g ko oc p", kp=P
    )                                                          # [128,2,16,4,128]

    sig = mybir.ActivationFunctionType.Sigmoid
    tanh = mybir.ActivationFunctionType.Tanh
    ko_hi = ko_limit or KO
    assert ko_hi % 2 == 0, "fp8 DoubleRow needs an even ko count"

    with tile.TileContext(nc) as tc:
        with (
            tc.tile_pool(name="zpool", bufs=1) as zpool,
            tc.tile_pool(name="cpool", bufs=1) as cpool,
            tc.tile_pool(name="bpool", bufs=1) as bpool,
            tc.tile_pool(name="wpool", bufs=1) as wpool,
            tc.tile_pool(name="gates", bufs=1) as gpool,
            tc.tile_pool(name="temps", bufs=2) as tpool,
            tc.tile_pool(name="psum", bufs=8, space="PSUM") as psum_pool,
        ):
            # explicit static slots (bufs=1 pools, distinct tags)
            z_slots = [
                zpool.tile([P, KO, B_L], BF16, tag=f"z{s}", name=f"z{s}")
                for s in range(ZS)
            ]
            zq_slots = [
                zpool.tile([P, KO, B_L], FP8, tag=f"zq{s}", name=f"zq{s}")
                for s in range(ZS)
            ]
            wb_slots = [
                wpool.tile([P, KO, 2 * P], BF16, tag=f"wb{s}", name=f"wb{s}")
                for s in range(WS)
            ]
            wq_slots = [
                wpool.tile([P, 2, KO, P], FP8, tag=f"wq{s}", name=f"wq{s}")
                for s in range(WS)
            ]
            c_slots = [
                cpool.tile([P, B_L], BF16, tag=f"c{s}", name=f"c{s}")
                for s in range(CS)
            ]
            b_sb = bpool.tile([P, OC, NG], F32, tag="b", name="b")

            def load_z(copy, quarter=None):
                # quarter loads flatten the DMA duty cycle (see docstring)
                if quarter is None:
                    nc.sync.dma_start(
                        z_slots[copy % ZS][:, :, :], zT_t[:, :, :]
                    )
                    nc.sync.dma_start(
                        zq_slots[copy % ZS][:, :, :], zQ_t[:, :, :]
                    )
                else:
                    ksl = slice(quarter * 4, (quarter + 1) * 4)
                    nc.sync.dma_start(
                        z_slots[copy % ZS][:, ksl, :], zT_t[:, ksl, :]
                    )
                    nc.sync.dma_start(
                        zq_slots[copy % ZS][:, ksl, :], zQ_t[:, ksl, :]
                    )

            def load_w(u):
                # weights for global unit u (copy u//OC, oc u%OC) -> slot u%WS
                nc.sync.dma_start(
                    wb_slots[u % WS][:, :, :], wB_t[:, :, u % OC, :]
                )
                nc.sync.dma_start(
                    wq_slots[u % WS][:, :, :, :], wQ_t[:, :, :, u % OC, :]
                )

            def load_c(u):
                nc.scalar.dma_start(c_slots[u % CS][:, :], cT_t[:, u % OC, :])

            def compute_unit(u):
                # ko-outer/gate-inner over one oc: the PSUM bank rotates on
                # every matmul so each matmul's drain overlaps the next
                # one's fill.  f/i ride fp8 DoubleRow on even ko rounds
                # (256 k per matmul), c/o ride bf16 every round.
                copy, oc = divmod(u, OC)
                z_sb = z_slots[copy % ZS]
                zq_sb = zq_slots[copy % ZS]
                wb_sb = wb_slots[u % WS]
                wq_sb = wq_slots[u % WS]
                c_sb = c_slots[u % CS]
                gate_sb = {}
                cf_sb = {}
                ps8 = {
                    (g, nb): psum_pool.tile([P, NT], F32, tag="ps", name="ps")
                    for g in GATES for nb in range(NB)
                }
                for ko in range(ko_hi):
                    if ko % 2 == 0:
                        for gi, g in enumerate(FP8_GATES):
                            for nb in range(NB):
                                nc.tensor.matmul(
                                    ps8[(g, nb)][:, :],
                                    lhsT=wq_sb[:, gi, ko:ko + 2, :],
                                    rhs=zq_sb[:, ko:ko + 2,
                                              nb * NT:(nb + 1) * NT],
                                    start=(ko == 0),
                                    stop=(ko == ko_hi - 2),
                                    perf_mode=mybir.MatmulPerfMode.DoubleRow,
                                )
                    for gi, g in enumerate(BF_GATES):
                        for nb in range(NB):
                            nc.tensor.matmul(
                                ps8[(g, nb)][:, :],
                                lhsT=wb_sb[:, ko, gi * P:(gi + 1) * P],
                                rhs=z_sb[:, ko, nb * NT:(nb + 1) * NT],
                                start=(ko == 0),
                                stop=(ko == ko_hi - 1),
                            )
                for gi, g in enumerate(GATES):
                    scale = QSCALE if g in FP8_GATES else 1.0
                    for nb in range(NB):
                        gt = gpool.tile(
                            [P, NT], F32, tag=f"gate_{g}_{nb}",
                            name=f"gate_{g}_{nb}",
                        )
                        nc.scalar.activation(
                            gt[:, :], ps8[(g, nb)][:, :],
                            tanh if g == "c" else sig,
                            bias=b_sb[:, oc, gi:gi + 1],
                            scale=scale,
                        )
                        gate_sb[(g, nb)] = gt
                    if g == "c":
                        # tanh(c*f + i*g) is independent of gate o — emit now
                        # so only mul+store remain after the last matmul
                        for nb in range(NB):
                            bsl = slice(nb * NT, (nb + 1) * NT)
                            cf = tpool.tile([P, NT], F32, tag="cf",
                                            name=f"cf_{nb}")
                            nc.vector.tensor_mul(
                                cf[:, :], c_sb[:, bsl],
                                gate_sb[("f", nb)][:, :],
                            )
                            ig = tpool.tile([P, NT], F32, tag="ig", name="ig")
                            nc.vector.tensor_mul(
                                ig[:, :], gate_sb[("i", nb)][:, :],
                                gate_sb[("c", nb)][:, :],
                            )
                            nc.vector.tensor_add(
                                cf[:, :], cf[:, :], ig[:, :]
                            )
                            nc.scalar.activation(cf[:, :], cf[:, :], tanh)
                            cf_sb[nb] = cf
                for nb in range(NB):
                    bsl = slice(nb * NT, (nb + 1) * NT)
                    ho = tpool.tile([P, NT], BF16, tag="ho", name="ho")
                    nc.vector.tensor_mul(
                        ho[:, :], cf_sb[nb][:, :], gate_sb[("o", nb)][:, :]
                    )
                    nc.scalar.dma_start(hT_t[:, oc, bsl], ho[:, :])

            # ---- prologue: first copy's working set -----------------------
            nc.scalar.dma_start(b_sb[:, :, :], bA[:, :, :])
            load_z(0)
            load_w(0)
            load_w(1)
            load_c(0)
            load_c(1)

            if loop_r:
                with tc.For_i(0, loop_r // NCOPIES, 1):
                    for u in range(NU):
                        # prefetch one quarter of copy c+1's z per unit —
                        # slot (c+1)%ZS was last read by copy c-1, already
                        # retired; all 4 quarters land across copy c
                        load_z(u // OC + 1, quarter=u % OC)
                        load_w(u + 2)          # slot (u+2)%WS, wraps to next
                        load_c(u + 2)
                        if u == 1:
                            nc.scalar.dma_start(b_sb[:, :, :], bA[:, :, :])
                        compute_unit(u)
            else:
                # one-shot: single copy, stream w/c two units ahead
                for u in range(OC):
                    if u + 2 < OC:
                        load_w(u + 2)
                        load_c(u + 2)
                    compute_unit(u)

    nc.finalize()
    return nc


def _get_nc():
    if "nc" not in _NC_CACHE:
        _NC_CACHE["nc"] = _build_nc()
    return _NC_CACHE["nc"]


def _shard_inputs(x, h, c, w_f, b_f, w_i, b_i, w_c, b_c, w_o, b_o):
    ws = {"f": w_f, "i": w_i, "c": w_c, "o": w_o}
    bz = {"f": b_f, "i": b_i, "c": b_c, "o": b_o}
    f32 = np.float32

    # per-out-group fused weight/bias shards (shared by the 4 batch groups)
    wB_sh = {}
    wQ_sh = {}
    bA_sh = {}
    for j in range(RO):
        cols = slice(j * O_L, (j + 1) * O_L)
        wB_sh[j] = np.ascontiguousarray(
            np.stack(
                [np.asarray(ws[g][:, cols], dtype=f32).reshape(K, OC, P)
                 for g in BF_GATES],
                axis=2,
            ).astype(NP_BF16)
        )
        wQ_sh[j] = np.ascontiguousarray(
            np.clip(
                np.stack(
                    [np.asarray(ws[g][:, cols], dtype=f32).reshape(K, OC, P)
                     for g in FP8_GATES],
                    axis=2,
                ) * SW,
                -240.0, 240.0,
            ).astype(NP_FP8)
        )
        bA_sh[j] = np.ascontiguousarray(
            np.stack(
                [np.asarray(bz[g], dtype=f32).reshape(-1)[cols]
                 .reshape(OC, P).T
                 for g in GATES],
                axis=2,
            )
        )

    in_maps = []
    for i in range(RB):
        rows = slice(i * B_L, (i + 1) * B_L)
        zTf = np.concatenate(
            [np.asarray(x[rows], dtype=f32), np.asarray(h[rows], dtype=f32)],
            axis=1,
        ).T  # [K, B_L] f32
        zT = np.ascontiguousarray(zTf.astype(NP_BF16))
        zQv = np.ascontiguousarray(
            np.clip(zTf * SZ, -240.0, 240.0).astype(NP_FP8)
        )
        for j in range(RO):
            cT = np.ascontiguousarray(
                np.asarray(c[rows, j * O_L:(j + 1) * O_L], dtype=f32).T
                .astype(NP_BF16)
            )
            in_maps.append(
                {"zT": zT, "zQ": zQv, "cT": cT,
                 "wB": wB_sh[j], "wQ": wQ_sh[j], "bA": bA_sh[j]}
            )
    return in_maps


def _run(in_maps, trace=False, trace_cores=None):
    global last_exec_time_ns
    nc = _get_nc()
    res = run_bass_kernel_spmd(
        nc, in_maps, list(range(RB * RO)),
        trace=trace, trace_cores=trace_cores,
    )
    if trace:
        last_exec_time_ns = res.exec_time_ns
    return res.results


def kernel(x, h, c, w_f, b_f, w_i, b_i, w_c, b_c, w_o, b_o):
    in_maps = _shard_inputs(
        x, h, c, w_f, b_f, w_i, b_i, w_c, b_c, w_o, b_o
    )
    results = _run(in_maps)
    out = np.empty((B_FULL, OUT), np.float32)
    for i in range(RB):
        for j in range(RO):
            shard = results[i * RO + j]["hT"]  # [O_L, B_L] bf16
            out[i * B_L:(i + 1) * B_L, j * O_L:(j + 1) * O_L] = (
                shard.astype(np.float32).T
            )
    return out


# revision 6
# speedup vs baseline: 1.3147x; 1.3147x over previous
"""LSTM layer kernel for Trainium2 (8 NeuronCores, Bass/Tile).

Reference computation (fp32):
    z = concat([x, h], axis=1)                 # [B, IN+OUT]
    f = sigmoid(z @ w_f + b_f)
    i = sigmoid(z @ w_i + b_i)
    g = tanh   (z @ w_c + b_c)
    o = sigmoid(z @ w_o + b_o)
    c_new = c * f + i * g
    h_new = tanh(c_new) * o                    # [B, OUT]

Shapes: B=4096, IN=OUT=1024, K=IN+OUT=2048.
Sharding (8 cores): 4 batch-groups x 2 output-column-groups; core (i, j)
computes h_new[i*1024:(i+1)*1024, j*512:(j+1)*512].  No collectives.

The PE is the sole bottleneck (ACT/DVE sit at ~13% duty), so the win over
the all-bf16 schedule is cutting PE cycles with fp8 DoubleRow matmuls:
e4m3 operands + perf_mode=DoubleRow contract 256 k-elements per 512-cycle
matmul (2 fp8 products per PE cell per cycle) — 2x bf16 FLOP rate.  Pure
e4m3 for all four gates fails the 2e-2 gate (measured 3.1e-2 end to end:
~2.4% RMS quantization noise per operand -> 3.8e-2 pre-activation noise),
but the gates' error sensitivities are skewed (candidate tanh 2.4e-2,
output 1.5e-2, forget 1.1e-2, input 0.8e-2 when quantized alone), so the
forget/input gates run in fp8 (8 DoubleRow matmuls each) while the
candidate/output gates stay bf16 (16 matmuls each): 48/64 of the bf16
slot count, sim rel err 1.37e-2 (hw-validated DoubleRow adds only ~1e-4
internal rounding noise).

fp8 scales are fixed powers of two (z*32, w*1024; e4m3 relative error is
scale-invariant, only clipping matters and |z|<7.5sigma, |w|<2.3x absmax
stay in +-240), undone by the activation's scale operand (2^-15) at zero
cost.  Quantization runs host-side from the f32 inputs (same class of
prep as the baseline's bf16 casts/transposes).

Device layout: contraction dim (k) and output-channel dim (o) sit on SBUF
partitions; zT [2048,1024] bf16 + zQ e4m3 are the moving operands, the
gate-fused weights (wB bf16 for c/o, wQ e4m3 for f/i) are stationary,
out = w.T @ zT lands in PSUM as [o, b] so bias/sigmoid/tanh run straight
out of PSUM.  wQ is laid out gate-outer ([kp, g, ko, p]) so the DoubleRow
lhsT slice [:, g, ko:ko+2, :] has the pair dim contiguous — the exact AP
shape validated on hw.  c rides bf16 (error contribution ~3e-3 rel,
negligible vs the fp8 noise) to keep SBUF at ~179KB/partition.

Scheduling (inherited from the bf16 baseline, measured there):
  - Tile's For_i puts an all-engine barrier on every back edge; the body
    holds NCOPIES=8 problem copies = 32 "oc units" so the ~2.2us barrier
    amortizes /8, and DMAs issued late in iteration n prefetch what n+1
    starts with (slots persist across the barrier).
  - Unit u's compute is preceded by the DMA for unit u+2's weights/c and
    one quarter of the next copy's z (both dtypes), flattening the DMA
    duty cycle (co-running DMA measurably slows the matmul stream).
  - Within a unit the matmul order is ko-outer/gate-inner: the PSUM bank
    rotates on every matmul (all 8 banks accumulate concurrently),
    overlapping each matmul's drain with the next one's fill.  The f/i
    DoubleRow matmuls ride the even ko rounds and stop at ko=14, so
    their ACT drains overlap the last c/o matmul rounds.
  - z/w loads ride the sync HWDGE ring; c/bias loads and h stores ride
    the scalar ring, so a store waiting on compute never head-of-line
    blocks a load.
  - Slot cycling is static: z/zQ 2 slots (copy parity), w 4 slots, c 4
    slots; unit u reads slot u%4 and the prefetch writes slot (u+2)%4,
    consistent across the loop wrap since 32%4==0.
"""

import numpy as np
import ml_dtypes

import concourse.bass as bass
import concourse.tile as tile
from concourse import bacc
from concourse import mybir
from concourse.bass_utils import run_bass_kernel_spmd

P = 128
B_FULL, IN, OUT = 4096, 1024, 1024
K = IN + OUT                 # 2048 contraction
RB, RO = 4, 2                # batch-shards x out-col-shards = 8 cores
B_L = B_FULL // RB           # 1024 batch rows per core
O_L = OUT // RO              # 512 out cols per core
KO = K // P                  # 16 k-chunks
OC = O_L // P                # 4 out chunks per core
NG = 4                       # gates
NT = 512                     # moving free dim per matmul (one PSUM bank)
NB = B_L // NT               # 2 batch tiles
NCOPIES = 8                  # kernel copies per hardware-loop iteration
ZS = 2                       # z slots (copy parity)
NU = NCOPIES * OC            # oc-units per body
WS = 4                       # w slots (divides NU)
CS = 4                       # c slots (divides NU)

SZ = 32.0                    # fp8 scale for z (power of 2: exact)
SW = 1024.0                  # fp8 scale for w_f/w_i
QSCALE = 1.0 / (SZ * SW)     # 2^-15, folded into the activation

F32 = mybir.dt.float32
BF16 = mybir.dt.bfloat16
FP8 = mybir.dt.float8e4
NP_BF16 = ml_dtypes.bfloat16
NP_FP8 = ml_dtypes.float8_e4m3   # TRN FP8_EXP4 bit pattern (max +-240)
GATES = ("f", "i", "c", "o")
FP8_GATES = ("f", "i")       # low-sensitivity gates -> e4m3 DoubleRow
BF_GATES = ("c", "o")        # high-sensitivity gates -> bf16

last_exec_time_ns = None

_NC_CACHE = {}


def _build_nc(loop_r=None, ko_limit=None):
    nc = bacc.Bacc()

    zT = nc.dram_tensor("zT", [K, B_L], BF16, kind="ExternalInput")
    zQ = nc.dram_tensor("zQ", [K, B_L], FP8, kind="ExternalInput")
    cT = nc.dram_tensor("cT", [O_L, B_L], BF16, kind="ExternalInput")
    # bf16 gate weights (c, o): [k, oc, gate, p] with o_local = oc*128 + p
    wB = nc.dram_tensor("wB", [K, OC, 2, P], BF16, kind="ExternalInput")
    # fp8 gate weights (f, i), pre-packed per-oc-contiguous gate-outer:
    # wQ[oc, kp, g, ko, p] so each unit's load is one flat copy
    wQ = nc.dram_tensor("wQ", [OC, P, 2, KO, P], FP8, kind="ExternalInput")
    # gate-fused biases: [p, oc, gate] in GATES order
    bA = nc.dram_tensor("bA", [P, OC, NG], F32, kind="ExternalInput")
    hT = nc.dram_tensor("hT", [O_L, B_L], BF16, kind="ExternalOutput")

    zT_t = zT[:, :].rearrange("(ko kp) b -> kp ko b", kp=P)    # [128,16,1024]
    zQ_t = zQ[:, :].rearrange("(ko kp) b -> kp ko b", kp=P)
    cT_t = cT[:, :].rearrange("(oc p) b -> p oc b", p=P)       # [128,4,1024]
    hT_t = hT[:, :].rearrange("(oc p) b -> p oc b", p=P)
    wB_t = wB[:, :, :, :].rearrange(
        "(ko kp) oc g p -> kp ko oc (g p)", kp=P
    )                                                          # [128,16,4,256]

    sig = mybir.ActivationFunctionType.Sigmoid
    tanh = mybir.ActivationFunctionType.Tanh
    ko_hi = ko_limit or KO
    assert ko_hi % 2 == 0, "fp8 DoubleRow needs an even ko count"

    with tile.TileContext(nc) as tc:
        with (
            tc.tile_pool(name="zpool", bufs=1) as zpool,
            tc.tile_pool(name="cpool", bufs=1) as cpool,
            tc.tile_pool(name="bpool", bufs=1) as bpool,
            tc.tile_pool(name="wpool", bufs=1) as wpool,
            tc.tile_pool(name="gates", bufs=1) as gpool,
            tc.tile_pool(name="temps", bufs=2) as tpool,
            tc.tile_pool(name="psum", bufs=8, space="PSUM") as psum_pool,
        ):
            # explicit static slots (bufs=1 pools, distinct tags)
            z_slots = [
                zpool.tile([P, KO, B_L], BF16, tag=f"z{s}", name=f"z{s}")
                for s in range(ZS)
            ]
            zq_slots = [
                zpool.tile([P, KO, B_L], FP8, tag=f"zq{s}", name=f"zq{s}")
                for s in range(ZS)
            ]
            wb_slots = [
                wpool.tile([P, KO, 2 * P], BF16, tag=f"wb{s}", name=f"wb{s}")
                for s in range(WS)
            ]
            wq_slots = [
                wpool.tile([P, 2, KO, P], FP8, tag=f"wq{s}", name=f"wq{s}")
                for s in range(WS)
            ]
            c_slots = [
                cpool.tile([P, B_L], BF16, tag=f"c{s}", name=f"c{s}")
                for s in range(CS)
            ]
            b_sb = bpool.tile([P, OC, NG], F32, tag="b", name="b")

            def load_z(copy, quarter=None):
                # quarter loads flatten the DMA duty cycle (see docstring)
                if quarter is None:
                    nc.sync.dma_start(
                        z_slots[copy % ZS][:, :, :], zT_t[:, :, :]
                    )
                    nc.sync.dma_start(
                        zq_slots[copy % ZS][:, :, :], zQ_t[:, :, :]
                    )
                else:
                    ksl = slice(quarter * 4, (quarter + 1) * 4)
                    nc.sync.dma_start(
                        z_slots[copy % ZS][:, ksl, :], zT_t[:, ksl, :]
                    )
                    nc.sync.dma_start(
                        zq_slots[copy % ZS][:, ksl, :], zQ_t[:, ksl, :]
                    )

            def load_w(u):
                # weights for global unit u (copy u//OC, oc u%OC) -> slot u%WS
                nc.sync.dma_start(
                    wb_slots[u % WS][:, :, :], wB_t[:, :, u % OC, :]
                )
                nc.sync.dma_start(
                    wq_slots[u % WS][:, :, :, :], wQ[u % OC, :, :, :, :]
                )

            def load_c(u):
                nc.scalar.dma_start(c_slots[u % CS][:, :], cT_t[:, u % OC, :])

            def compute_unit(u):
                # ko-outer/gate-inner over one oc: the PSUM bank rotates on
                # every matmul so each matmul's drain overlaps the next
                # one's fill.  f/i ride fp8 DoubleRow on even ko rounds
                # (256 k per matmul), c/o ride bf16 every round.
                copy, oc = divmod(u, OC)
                z_sb = z_slots[copy % ZS]
                zq_sb = zq_slots[copy % ZS]
                wb_sb = wb_slots[u % WS]
                wq_sb = wq_slots[u % WS]
                c_sb = c_slots[u % CS]
                gate_sb = {}
                cf_sb = {}
                ps8 = {
                    (g, nb): psum_pool.tile([P, NT], F32, tag="ps", name="ps")
                    for g in GATES for nb in range(NB)
                }
                for ko in range(ko_hi):
                    if ko % 2 == 0:
                        for gi, g in enumerate(FP8_GATES):
                            for nb in range(NB):
                                nc.tensor.matmul(
                                    ps8[(g, nb)][:, :],
                                    lhsT=wq_sb[:, gi, ko:ko + 2, :],
                                    rhs=zq_sb[:, ko:ko + 2,
                                              nb * NT:(nb + 1) * NT],
                                    start=(ko == 0),
                                    stop=(ko == ko_hi - 2),
                                    perf_mode=mybir.MatmulPerfMode.DoubleRow,
                                )
                    for gi, g in enumerate(BF_GATES):
                        for nb in range(NB):
                            nc.tensor.matmul(
                                ps8[(g, nb)][:, :],
                                lhsT=wb_sb[:, ko, gi * P:(gi + 1) * P],
                                rhs=z_sb[:, ko, nb * NT:(nb + 1) * NT],
                                start=(ko == 0),
                                stop=(ko == ko_hi - 1),
                            )
                for gi, g in enumerate(GATES):
                    scale = QSCALE if g in FP8_GATES else 1.0
                    for nb in range(NB):
                        gt = gpool.tile(
                            [P, NT], F32, tag=f"gate_{g}_{nb}",
                            name=f"gate_{g}_{nb}",
                        )
                        nc.scalar.activation(
                            gt[:, :], ps8[(g, nb)][:, :],
                            tanh if g == "c" else sig,
                            bias=b_sb[:, oc, gi:gi + 1],
                            scale=scale,
                        )
                        gate_sb[(g, nb)] = gt
                    if g == "c":
                        # tanh(c*f + i*g) is independent of gate o — emit now
                        # so only mul+store remain after the last matmul
                        for nb in range(NB):
                            bsl = slice(nb * NT, (nb + 1) * NT)
                            cf = tpool.tile([P, NT], F32, tag="cf",
                                            name=f"cf_{nb}")
                            nc.vector.tensor_mul(
                                cf[:, :], c_sb[:, bsl],
                                gate_sb[("f", nb)][:, :],
                            )
                            ig = tpool.tile([P, NT], F32, tag="ig", name="ig")
                            nc.vector.tensor_mul(
                                ig[:, :], gate_sb[("i", nb)][:, :],
                                gate_sb[("c", nb)][:, :],
                            )
                            nc.vector.tensor_add(
                                cf[:, :], cf[:, :], ig[:, :]
                            )
                            nc.scalar.activation(cf[:, :], cf[:, :], tanh)
                            cf_sb[nb] = cf
                for nb in range(NB):
                    bsl = slice(nb * NT, (nb + 1) * NT)
                    ho = tpool.tile([P, NT], BF16, tag="ho", name="ho")
                    nc.vector.tensor_mul(
                        ho[:, :], cf_sb[nb][:, :], gate_sb[("o", nb)][:, :]
                    )
                    nc.scalar.dma_start(hT_t[:, oc, bsl], ho[:, :])

            # ---- prologue: first copy's working set -----------------------
            nc.scalar.dma_start(b_sb[:, :, :], bA[:, :, :])
            load_z(0)
            load_w(0)
            load_w(1)
            load_c(0)
            load_c(1)

            if loop_r:
                with tc.For_i(0, loop_r // NCOPIES, 1):
                    for u in range(NU):
                        # prefetch one quarter of copy c+1's z per unit —
                        # slot (c+1)%ZS was last read by copy c-1, already
                        # retired; all 4 quarters land across copy c
                        load_z(u // OC + 1, quarter=u % OC)
                        load_w(u + 2)          # slot (u+2)%WS, wraps to next
                        load_c(u + 2)
                        if u == 1:
                            nc.scalar.dma_start(b_sb[:, :, :], bA[:, :, :])
                        compute_unit(u)
            else:
                # one-shot: single copy, stream w/c two units ahead
                for u in range(OC):
                    if u + 2 < OC:
                        load_w(u + 2)
                        load_c(u + 2)
                    compute_unit(u)

    nc.finalize()
    return nc


def _get_nc():
    if "nc" not in _NC_CACHE:
        _NC_CACHE["nc"] = _build_nc()
    return _NC_CACHE["nc"]


def _shard_inputs(x, h, c, w_f, b_f, w_i, b_i, w_c, b_c, w_o, b_o):
    ws = {"f": w_f, "i": w_i, "c": w_c, "o": w_o}
    bz = {"f": b_f, "i": b_i, "c": b_c, "o": b_o}
    f32 = np.float32

    # per-out-group fused weight/bias shards (shared by the 4 batch groups)
    wB_sh = {}
    wQ_sh = {}
    bA_sh = {}
    for j in range(RO):
        cols = slice(j * O_L, (j + 1) * O_L)
        wB_sh[j] = np.ascontiguousarray(
            np.stack(
                [np.asarray(ws[g][:, cols], dtype=f32).reshape(K, OC, P)
                 for g in BF_GATES],
                axis=2,
            ).astype(NP_BF16)
        )
        # wQ[oc, kp, g, ko, p] <- w_g[ko*128+kp, j*O_L + oc*128 + p]
        wq = np.stack(
            [np.asarray(ws[g][:, cols], dtype=f32) for g in FP8_GATES],
            axis=1,
        )  # [K, 2, O_L]
        wq = wq.reshape(KO, P, 2, OC, P).transpose(3, 1, 2, 0, 4)
        wQ_sh[j] = np.ascontiguousarray(
            np.clip(wq * SW, -240.0, 240.0).astype(NP_FP8)
        )
        bA_sh[j] = np.ascontiguousarray(
            np.stack(
                [np.asarray(bz[g], dtype=f32).reshape(-1)[cols]
                 .reshape(OC, P).T
                 for g in GATES],
                axis=2,
            )
        )

    in_maps = []
    for i in range(RB):
        rows = slice(i * B_L, (i + 1) * B_L)
        zTf = np.concatenate(
            [np.asarray(x[rows], dtype=f32), np.asarray(h[rows], dtype=f32)],
            axis=1,
        ).T  # [K, B_L] f32
        zT = np.ascontiguousarray(zTf.astype(NP_BF16))
        zQv = np.ascontiguousarray(
            np.clip(zTf * SZ, -240.0, 240.0).astype(NP_FP8)
        )
        for j in range(RO):
            cT = np.ascontiguousarray(
                np.asarray(c[rows, j * O_L:(j + 1) * O_L], dtype=f32).T
                .astype(NP_BF16)
            )
            in_maps.append(
                {"zT": zT, "zQ": zQv, "cT": cT,
                 "wB": wB_sh[j], "wQ": wQ_sh[j], "bA": bA_sh[j]}
            )
    return in_maps


def _run(in_maps, trace=False, trace_cores=None):
    global last_exec_time_ns
    nc = _get_nc()
    res = run_bass_kernel_spmd(
        nc, in_maps, list(range(RB * RO)),
        trace=trace, trace_cores=trace_cores,
    )
    if trace:
        last_exec_time_ns = res.exec_time_ns
    return res.results


def kernel(x, h, c, w_f, b_f, w_i, b_i, w_c, b_c, w_o, b_o):
    in_maps = _shard_inputs(
        x, h, c, w_f, b_f, w_i, b_i, w_c, b_c, w_o, b_o
    )
    results = _run(in_maps)
    out = np.empty((B_FULL, OUT), np.float32)
    for i in range(RB):
        for j in range(RO):
            shard = results[i * RO + j]["hT"]  # [O_L, B_L] bf16
            out[i * B_L:(i + 1) * B_L, j * O_L:(j + 1) * O_L] = (
                shard.astype(np.float32).T
            )
    return out


# revision 13
# speedup vs baseline: 1.4520x; 1.1045x over previous
"""LSTM layer kernel for Trainium2 (8 NeuronCores, Bass/Tile).

Reference computation (fp32):
    z = concat([x, h], axis=1)                 # [B, IN+OUT]
    f = sigmoid(z @ w_f + b_f)
    i = sigmoid(z @ w_i + b_i)
    g = tanh   (z @ w_c + b_c)
    o = sigmoid(z @ w_o + b_o)
    c_new = c * f + i * g
    h_new = tanh(c_new) * o                    # [B, OUT]

Shapes: B=4096, IN=OUT=1024, K=IN+OUT=2048.
Sharding (8 cores): 4 batch-groups x 2 output-column-groups; core (i, j)
computes h_new[i*1024:(i+1)*1024, j*512:(j+1)*512].  No collectives.

The PE is the sole bottleneck (ACT/DVE sit at ~13% duty), so the win over
the all-bf16 schedule is cutting PE cycles with fp8 DoubleRow matmuls:
e4m3 operands + perf_mode=DoubleRow contract 256 k-elements per 512-cycle
matmul (2 fp8 products per PE cell per cycle) — 2x bf16 FLOP rate.  Pure
e4m3 for all four gates fails the 2e-2 gate (measured 3.1e-2 end to end:
~2.4% RMS quantization noise per operand -> 3.8e-2 pre-activation noise),
but the gates' error sensitivities are skewed (candidate tanh 2.4e-2,
output 1.5e-2, forget 1.1e-2, input 0.8e-2 when quantized alone), so the
forget/input gates run in fp8 (8 DoubleRow matmuls each) while the
candidate/output gates stay bf16 (16 matmuls each): 48/64 of the bf16
slot count, sim rel err 1.37e-2 (hw-validated DoubleRow adds only ~1e-4
internal rounding noise).

fp8 scales are fixed powers of two (z*32, w*1024; e4m3 relative error is
scale-invariant, only clipping matters and |z|<7.5sigma, |w|<2.3x absmax
stay in +-240), undone by the activation's scale operand (2^-15) at zero
cost.  Quantization runs host-side from the f32 inputs (same class of
prep as the baseline's bf16 casts/transposes).

Device layout: contraction dim (k) and output-channel dim (o) sit on SBUF
partitions; zT [2048,1024] bf16 + zQ e4m3 are the moving operands, the
gate-fused weights (wB bf16 for c/o, wQ e4m3 for f/i) are stationary,
out = w.T @ zT lands in PSUM as [o, b] so bias/sigmoid/tanh run straight
out of PSUM.  wQ is laid out gate-outer ([kp, g, ko, p]) so the DoubleRow
lhsT slice [:, g, ko:ko+2, :] has the pair dim contiguous — the exact AP
shape validated on hw.  c rides bf16 (error contribution ~3e-3 rel,
negligible vs the fp8 noise) to keep SBUF at ~179KB/partition.

Scheduling (inherited from the bf16 baseline, measured there):
  - Tile's For_i puts an all-engine barrier on every back edge; the body
    holds NCOPIES=8 problem copies = 32 "oc units" so the ~2.2us barrier
    amortizes /8, and DMAs issued late in iteration n prefetch what n+1
    starts with (slots persist across the barrier).
  - Unit u's compute is preceded by the DMA for unit u+2's weights/c and
    one quarter of the next copy's z (both dtypes), flattening the DMA
    duty cycle (co-running DMA measurably slows the matmul stream).
  - Within a unit the matmul order is ko-outer/gate-inner: the PSUM bank
    rotates on every matmul (all 8 banks accumulate concurrently),
    overlapping each matmul's drain with the next one's fill.  The f/i
    DoubleRow matmuls ride the even ko rounds and stop at ko=14, so
    their ACT drains overlap the last c/o matmul rounds.
  - z/w loads ride the sync HWDGE ring; c/bias loads and h stores ride
    the scalar ring, so a store waiting on compute never head-of-line
    blocks a load.
  - Slot cycling is static: z/zQ 2 slots (copy parity), w 4 slots, c 4
    slots; unit u reads slot u%4 and the prefetch writes slot (u+2)%4,
    consistent across the loop wrap since 32%4==0.
"""

import numpy as np
import ml_dtypes

import concourse.bass as bass
import concourse.tile as tile
from concourse import bacc
from concourse import mybir
from concourse.bass_utils import run_bass_kernel_spmd

P = 128
B_FULL, IN, OUT = 4096, 1024, 1024
K = IN + OUT                 # 2048 contraction
RB, RO = 4, 2                # batch-shards x out-col-shards = 8 cores
B_L = B_FULL // RB           # 1024 batch rows per core
O_L = OUT // RO              # 512 out cols per core
KO = K // P                  # 16 k-chunks
OC = O_L // P                # 4 out chunks per core
NG = 4                       # gates
NT = 512                     # moving free dim per matmul (one PSUM bank)
NB = B_L // NT               # 2 batch tiles
NCOPIES = 8                  # kernel copies per hardware-loop iteration
ZS = 2                       # z slots (copy parity)
NU = NCOPIES * OC            # oc-units per body
WS = 4                       # w slots (divides NU)
CS = 4                       # c slots (divides NU)

SZ = 32.0                    # fp8 scale for z (power of 2: exact)
SW = 1024.0                  # fp8 scale for w_f/w_i/w_o
QSCALE = 1.0 / (SZ * SW)     # 2^-15, folded into the activation
MIX_O = 8                    # o-gate ko chunks in fp8 (rest bf16)
QU = 2 * KO + MIX_O          # fp8 weight units: f 16, i 16, o 8
BU = KO + (KO - MIX_O)       # bf16 weight units: c 16, o 8

F32 = mybir.dt.float32
BF16 = mybir.dt.bfloat16
FP8 = mybir.dt.float8e4
NP_BF16 = ml_dtypes.bfloat16
NP_FP8 = ml_dtypes.float8_e4m3   # TRN FP8_EXP4 bit pattern (max +-240)
GATES = ("f", "i", "c", "o")
FP8_GATES = ("f", "i")       # low-sensitivity gates -> e4m3 DoubleRow
BF_GATES = ("c", "o")        # high-sensitivity gates -> bf16

last_exec_time_ns = None

_NC_CACHE = {}


def _build_nc(loop_r=None, ko_limit=None):
    nc = bacc.Bacc()

    zT = nc.dram_tensor("zT", [K, B_L], BF16, kind="ExternalInput")
    zQ = nc.dram_tensor("zQ", [K, B_L], FP8, kind="ExternalInput")
    cT = nc.dram_tensor("cT", [O_L, B_L], BF16, kind="ExternalInput")
    # bf16 weight units, flat per-oc-contiguous: [oc, kp, u, p] with
    # u 0..15 = gate c ko 0..15, u 16..23 = gate o ko 8..15 (x 2^15)
    wB = nc.dram_tensor("wB", [OC, P, BU, P], BF16, kind="ExternalInput")
    # fp8 weight units: u 0..15 = f, 16..31 = i, 32..39 = o ko 0..7
    wQ = nc.dram_tensor("wQ", [OC, P, QU, P], FP8, kind="ExternalInput")
    # gate-fused biases: [p, oc, gate] in GATES order
    bA = nc.dram_tensor("bA", [P, OC, NG], F32, kind="ExternalInput")
    hT = nc.dram_tensor("hT", [O_L, B_L], BF16, kind="ExternalOutput")

    zT_t = zT[:, :].rearrange("(ko kp) b -> kp ko b", kp=P)    # [128,16,1024]
    zQ_t = zQ[:, :].rearrange("(ko kp) b -> kp ko b", kp=P)
    cT_t = cT[:, :].rearrange("(oc p) b -> p oc b", p=P)       # [128,4,1024]
    hT_t = hT[:, :].rearrange("(oc p) b -> p oc b", p=P)

    sig = mybir.ActivationFunctionType.Sigmoid
    tanh = mybir.ActivationFunctionType.Tanh
    ko_hi = ko_limit or KO
    assert ko_hi % 2 == 0, "fp8 DoubleRow needs an even ko count"

    with tile.TileContext(nc) as tc:
        with (
            tc.tile_pool(name="zpool", bufs=1) as zpool,
            tc.tile_pool(name="cpool", bufs=1) as cpool,
            tc.tile_pool(name="bpool", bufs=1) as bpool,
            tc.tile_pool(name="wpool", bufs=1) as wpool,
            tc.tile_pool(name="gates", bufs=1) as gpool,
            tc.tile_pool(name="temps", bufs=2) as tpool,
            tc.tile_pool(name="psum", bufs=8, space="PSUM") as psum_pool,
        ):
            # explicit static slots (bufs=1 pools, distinct tags)
            z_slots = [
                zpool.tile([P, KO, B_L], BF16, tag=f"z{s}", name=f"z{s}")
                for s in range(ZS)
            ]
            zq_slots = [
                zpool.tile([P, KO, B_L], FP8, tag=f"zq{s}", name=f"zq{s}")
                for s in range(ZS)
            ]
            wb_slots = [
                wpool.tile([P, BU, P], BF16, tag=f"wb{s}", name=f"wb{s}")
                for s in range(WS)
            ]
            wq_slots = [
                wpool.tile([P, QU, P], FP8, tag=f"wq{s}", name=f"wq{s}")
                for s in range(WS)
            ]
            c_slots = [
                cpool.tile([P, B_L], BF16, tag=f"c{s}", name=f"c{s}")
                for s in range(CS)
            ]
            b_sb = bpool.tile([P, OC, NG], F32, tag="b", name="b")

            def load_z(copy, quarter=None):
                # quarter loads flatten the DMA duty cycle (see docstring)
                if quarter is None:
                    nc.sync.dma_start(
                        z_slots[copy % ZS][:, :, :], zT_t[:, :, :]
                    )
                    nc.sync.dma_start(
                        zq_slots[copy % ZS][:, :, :], zQ_t[:, :, :]
                    )
                else:
                    ksl = slice(quarter * 4, (quarter + 1) * 4)
                    nc.sync.dma_start(
                        z_slots[copy % ZS][:, ksl, :], zT_t[:, ksl, :]
                    )
                    nc.sync.dma_start(
                        zq_slots[copy % ZS][:, ksl, :], zQ_t[:, ksl, :]
                    )

            def load_w(u):
                # weights for global unit u (copy u//OC, oc u%OC) -> slot u%WS
                nc.sync.dma_start(
                    wb_slots[u % WS][:, :, :], wB[u % OC, :, :, :]
                )
                nc.sync.dma_start(
                    wq_slots[u % WS][:, :, :], wQ[u % OC, :, :, :]
                )

            def load_c(u):
                nc.scalar.dma_start(c_slots[u % CS][:, :], cT_t[:, u % OC, :])

            def compute_unit(u):
                # ko-outer/gate-inner over one oc: the PSUM bank rotates on
                # every matmul so each matmul's drain overlaps the next
                # one's fill.  f/i ride fp8 DoubleRow on even ko rounds
                # (256 k per matmul), c/o ride bf16 every round.
                copy, oc = divmod(u, OC)
                z_sb = z_slots[copy % ZS]
                zq_sb = zq_slots[copy % ZS]
                wb_sb = wb_slots[u % WS]
                wq_sb = wq_slots[u % WS]
                c_sb = c_slots[u % CS]
                gate_sb = {}
                cf_sb = {}
                ps8 = {
                    (g, nb): psum_pool.tile([P, NT], F32, tag="ps", name="ps")
                    for g in GATES for nb in range(NB)
                }
                def mm_fp8(g, uoff, ko, start, stop):
                    for nb in range(NB):
                        nc.tensor.matmul(
                            ps8[(g, nb)][:, :],
                            lhsT=wq_sb[:, uoff + ko:uoff + ko + 2, :],
                            rhs=zq_sb[:, ko:ko + 2, nb * NT:(nb + 1) * NT],
                            start=start,
                            stop=stop,
                            perf_mode=mybir.MatmulPerfMode.DoubleRow,
                        )

                def mm_bf16(g, wu, ko, start, stop):
                    for nb in range(NB):
                        nc.tensor.matmul(
                            ps8[(g, nb)][:, :],
                            lhsT=wb_sb[:, wu, :],
                            rhs=z_sb[:, ko, nb * NT:(nb + 1) * NT],
                            start=start,
                            stop=stop,
                        )

                for ko in range(ko_hi):
                    if ko % 2 == 0:
                        mm_fp8("f", 0, ko, ko == 0, ko == ko_hi - 2)
                        mm_fp8("i", KO, ko, ko == 0, ko == ko_hi - 2)
                        if ko < MIX_O:
                            mm_fp8("o", 2 * KO, ko, ko == 0, False)
                    mm_bf16("c", ko, ko, ko == 0, ko == ko_hi - 1)
                    if ko >= MIX_O:
                        mm_bf16("o", KO + ko - MIX_O, ko,
                                False, ko == ko_hi - 1)
                for gi, g in enumerate(GATES):
                    scale = 1.0 if g == "c" else QSCALE
                    for nb in range(NB):
                        gt = gpool.tile(
                            [P, NT], F32, tag=f"gate_{g}_{nb}",
                            name=f"gate_{g}_{nb}",
                        )
                        nc.scalar.activation(
                            gt[:, :], ps8[(g, nb)][:, :],
                            tanh if g == "c" else sig,
                            bias=b_sb[:, oc, gi:gi + 1],
                            scale=scale,
                        )
                        gate_sb[(g, nb)] = gt
                    if g == "c":
                        # tanh(c*f + i*g) is independent of gate o — emit now
                        # so only mul+store remain after the last matmul
                        for nb in range(NB):
                            bsl = slice(nb * NT, (nb + 1) * NT)
                            cf = tpool.tile([P, NT], F32, tag="cf",
                                            name=f"cf_{nb}")
                            nc.vector.tensor_mul(
                                cf[:, :], c_sb[:, bsl],
                                gate_sb[("f", nb)][:, :],
                            )
                            ig = tpool.tile([P, NT], F32, tag="ig", name="ig")
                            nc.vector.tensor_mul(
                                ig[:, :], gate_sb[("i", nb)][:, :],
                                gate_sb[("c", nb)][:, :],
                            )
                            nc.vector.tensor_add(
                                cf[:, :], cf[:, :], ig[:, :]
                            )
                            nc.scalar.activation(cf[:, :], cf[:, :], tanh)
                            cf_sb[nb] = cf
                for nb in range(NB):
                    bsl = slice(nb * NT, (nb + 1) * NT)
                    ho = tpool.tile([P, NT], BF16, tag="ho", name="ho")
                    nc.vector.tensor_mul(
                        ho[:, :], cf_sb[nb][:, :], gate_sb[("o", nb)][:, :]
                    )
                    nc.scalar.dma_start(hT_t[:, oc, bsl], ho[:, :])

            # ---- prologue: first copy's working set -----------------------
            nc.scalar.dma_start(b_sb[:, :, :], bA[:, :, :])
            load_z(0)
            load_w(0)
            load_w(1)
            load_c(0)
            load_c(1)

            if loop_r:
                with tc.For_i(0, loop_r // NCOPIES, 1):
                    for u in range(NU):
                        # prefetch one quarter of copy c+1's z per unit —
                        # slot (c+1)%ZS was last read by copy c-1, already
                        # retired; all 4 quarters land across copy c
                        load_z(u // OC + 1, quarter=u % OC)
                        load_w(u + 2)          # slot (u+2)%WS, wraps to next
                        load_c(u + 2)
                        if u == 1:
                            nc.scalar.dma_start(b_sb[:, :, :], bA[:, :, :])
                        compute_unit(u)
            else:
                # one-shot: single copy, stream w/c two units ahead
                for u in range(OC):
                    if u + 2 < OC:
                        load_w(u + 2)
                        load_c(u + 2)
                    compute_unit(u)

    nc.finalize()
    return nc


def _get_nc():
    if "nc" not in _NC_CACHE:
        _NC_CACHE["nc"] = _build_nc()
    return _NC_CACHE["nc"]


def _shard_inputs(x, h, c, w_f, b_f, w_i, b_i, w_c, b_c, w_o, b_o):
    ws = {"f": w_f, "i": w_i, "c": w_c, "o": w_o}
    bz = {"f": b_f, "i": b_i, "c": b_c, "o": b_o}
    f32 = np.float32

    # per-out-group fused weight/bias shards (shared by the 4 batch groups)
    wB_sh = {}
    wQ_sh = {}
    bA_sh = {}
    for j in range(RO):
        cols = slice(j * O_L, (j + 1) * O_L)

        def units(g):
            # [KO, kp, OC, p] f32 view of w_g's column shard
            return (
                np.asarray(ws[g][:, cols], dtype=f32)
                .reshape(KO, P, OC, P)
            )

        # bf16 units: c ko 0..15, then o ko MIX_O..15 scaled by 2^15 so its
        # PSUM partial sits on the same scale as the fp8 o partial
        wb = np.concatenate(
            [units("c"), units("o")[MIX_O:] * (SZ * SW)], axis=0
        )  # [BU, kp, OC, p]
        wB_sh[j] = np.ascontiguousarray(
            wb.transpose(2, 1, 0, 3).astype(NP_BF16)
        )
        # fp8 units: f ko 0..15, i ko 0..15, o ko 0..MIX_O-1 (x SW)
        wq = np.concatenate(
            [units("f"), units("i"), units("o")[:MIX_O]], axis=0
        )  # [QU, kp, OC, p]
        wQ_sh[j] = np.ascontiguousarray(
            np.clip(wq.transpose(2, 1, 0, 3) * SW, -240.0, 240.0)
            .astype(NP_FP8)
        )
        bA_sh[j] = np.ascontiguousarray(
            np.stack(
                [np.asarray(bz[g], dtype=f32).reshape(-1)[cols]
                 .reshape(OC, P).T
                 for g in GATES],
                axis=2,
            )
        )

    in_maps = []
    for i in range(RB):
        rows = slice(i * B_L, (i + 1) * B_L)
        zTf = np.concatenate(
            [np.asarray(x[rows], dtype=f32), np.asarray(h[rows], dtype=f32)],
            axis=1,
        ).T  # [K, B_L] f32
        zT = np.ascontiguousarray(zTf.astype(NP_BF16))
        zQv = np.ascontiguousarray(
            np.clip(zTf * SZ, -240.0, 240.0).astype(NP_FP8)
        )
        for j in range(RO):
            cT = np.ascontiguousarray(
                np.asarray(c[rows, j * O_L:(j + 1) * O_L], dtype=f32).T
                .astype(NP_BF16)
            )
            in_maps.append(
                {"zT": zT, "zQ": zQv, "cT": cT,
                 "wB": wB_sh[j], "wQ": wQ_sh[j], "bA": bA_sh[j]}
            )
    return in_maps


def _run(in_maps, trace=False, trace_cores=None):
    global last_exec_time_ns
    nc = _get_nc()
    res = run_bass_kernel_spmd(
        nc, in_maps, list(range(RB * RO)),
        trace=trace, trace_cores=trace_cores,
    )
    if trace:
        last_exec_time_ns = res.exec_time_ns
    return res.results


def kernel(x, h, c, w_f, b_f, w_i, b_i, w_c, b_c, w_o, b_o):
    in_maps = _shard_inputs(
        x, h, c, w_f, b_f, w_i, b_i, w_c, b_c, w_o, b_o
    )
    results = _run(in_maps)
    out = np.empty((B_FULL, OUT), np.float32)
    for i in range(RB):
        for j in range(RO):
            shard = results[i * RO + j]["hT"]  # [O_L, B_L] bf16
            out[i * B_L:(i + 1) * B_L, j * O_L:(j + 1) * O_L] = (
                shard.astype(np.float32).T
            )
    return out


# revision 15
# speedup vs baseline: 1.5382x; 1.0593x over previous
"""LSTM layer kernel for Trainium2 (8 NeuronCores, Bass/Tile).

Reference computation (fp32):
    z = concat([x, h], axis=1)                 # [B, IN+OUT]
    f = sigmoid(z @ w_f + b_f)
    i = sigmoid(z @ w_i + b_i)
    g = tanh   (z @ w_c + b_c)
    o = sigmoid(z @ w_o + b_o)
    c_new = c * f + i * g
    h_new = tanh(c_new) * o                    # [B, OUT]

Shapes: B=4096, IN=OUT=1024, K=IN+OUT=2048.
Sharding (8 cores): 4 batch-groups x 2 output-column-groups; core (i, j)
computes h_new[i*1024:(i+1)*1024, j*512:(j+1)*512].  No collectives.

The PE is the sole bottleneck (ACT/DVE sit at ~13% duty), so the win over
the all-bf16 schedule is cutting PE cycles with fp8 DoubleRow matmuls:
e4m3 operands + perf_mode=DoubleRow contract 256 k-elements per 512-cycle
matmul (2 fp8 products per PE cell per cycle) — 2x bf16 FLOP rate.  Pure
e4m3 for all four gates fails the 2e-2 gate (measured 3.1e-2 end to end:
~2.4% RMS quantization noise per operand -> 3.8e-2 pre-activation noise),
but the gates' error sensitivities are skewed (candidate tanh 2.4e-2,
output 1.5e-2, forget 1.1e-2, input 0.8e-2 when quantized alone).
Allocation chosen from a numpy sim that tracks hw to ~4 digits (verified
twice on device): forget/input gates fully fp8 (8 DoubleRow matmuls
each), candidate stays bf16 (16), output runs its first MIX_O=12 ko
chunks fp8 + last 4 bf16 (6 DoubleRow + 4 bf16 matmuls): 42/64 of the
bf16 slot count, rel err 1.864e-2 on hw (seed-to-seed spread measured
<0.01% — the margin is deterministic, not statistical; MIX_O=16 would be
2.00e-2, right at the gate).

fp8 scales are fixed powers of two (z*32, w*1024; e4m3 relative error is
scale-invariant — a sweep moved rel err <0.2% — only clipping matters
and |z|<7.5sigma, |w|<2.3x absmax stay in +-240), undone by the
activation's scale operand (2^-15) at zero cost.  The o-gate's bf16
weight chunks are pre-scaled by 2^15 (exact exponent shift) so its bf16
PSUM partial lands on the same scale as its fp8 partial.  Quantization
runs host-side from the f32 inputs (same class of prep as the baseline's
bf16 casts/transposes).

Device layout: contraction dim (k) and output-channel dim (o) sit on SBUF
partitions; zT [2048,1024] bf16 + zQ e4m3 are the moving operands, the
gate-fused weights (wB bf16 for c/o, wQ e4m3 for f/i) are stationary,
out = w.T @ zT lands in PSUM as [o, b] so bias/sigmoid/tanh run straight
out of PSUM.  wQ is laid out gate-outer ([kp, g, ko, p]) so the DoubleRow
lhsT slice [:, g, ko:ko+2, :] has the pair dim contiguous — the exact AP
shape validated on hw.  c rides bf16 (error contribution ~3e-3 rel,
negligible vs the fp8 noise) to keep SBUF at ~179KB/partition.

Scheduling (inherited from the bf16 baseline, measured there):
  - Tile's For_i puts an all-engine barrier on every back edge; the body
    holds NCOPIES=8 problem copies = 32 "oc units" so the ~2.2us barrier
    amortizes /8, and DMAs issued late in iteration n prefetch what n+1
    starts with (slots persist across the barrier).
  - Unit u's compute is preceded by the DMA for unit u+2's weights/c and
    one quarter of the next copy's z (both dtypes), flattening the DMA
    duty cycle (co-running DMA measurably slows the matmul stream).
  - Within a unit the matmul order is ko-outer/gate-inner: the PSUM bank
    rotates on every matmul (all 8 banks accumulate concurrently),
    overlapping each matmul's drain with the next one's fill.  The f/i
    DoubleRow matmuls ride the even ko rounds and stop at ko=14, so
    their ACT drains overlap the last c/o matmul rounds.
  - z/w loads ride the sync HWDGE ring; c/bias loads and h stores ride
    the scalar ring, so a store waiting on compute never head-of-line
    blocks a load.
  - Slot cycling is static: z/zQ 2 slots (copy parity), w 4 slots, c 4
    slots; unit u reads slot u%4 and the prefetch writes slot (u+2)%4,
    consistent across the loop wrap since 32%4==0.
"""

import numpy as np
import ml_dtypes

import concourse.bass as bass
import concourse.tile as tile
from concourse import bacc
from concourse import mybir
from concourse.bass_utils import run_bass_kernel_spmd

P = 128
B_FULL, IN, OUT = 4096, 1024, 1024
K = IN + OUT                 # 2048 contraction
RB, RO = 4, 2                # batch-shards x out-col-shards = 8 cores
B_L = B_FULL // RB           # 1024 batch rows per core
O_L = OUT // RO              # 512 out cols per core
KO = K // P                  # 16 k-chunks
OC = O_L // P                # 4 out chunks per core
NG = 4                       # gates
NT = 512                     # moving free dim per matmul (one PSUM bank)
NB = B_L // NT               # 2 batch tiles
NCOPIES = 8                  # kernel copies per hardware-loop iteration
ZS = 2                       # z slots (copy parity)
NU = NCOPIES * OC            # oc-units per body
WS = 4                       # w slots (divides NU)
CS = 4                       # c slots (divides NU)

SZ = 32.0                    # fp8 scale for z (power of 2: exact)
SW = 1024.0                  # fp8 scale for w_f/w_i/w_o
QSCALE = 1.0 / (SZ * SW)     # 2^-15, folded into the activation
MIX_O = 12                   # o-gate ko chunks in fp8 (rest bf16)
QU = 2 * KO + MIX_O          # fp8 weight units: f 16, i 16, o 8
BU = KO + (KO - MIX_O)       # bf16 weight units: c 16, o 8

F32 = mybir.dt.float32
BF16 = mybir.dt.bfloat16
FP8 = mybir.dt.float8e4
NP_BF16 = ml_dtypes.bfloat16
NP_FP8 = ml_dtypes.float8_e4m3   # TRN FP8_EXP4 bit pattern (max +-240)
GATES = ("f", "i", "c", "o")
FP8_GATES = ("f", "i")       # low-sensitivity gates -> e4m3 DoubleRow
BF_GATES = ("c", "o")        # high-sensitivity gates -> bf16

last_exec_time_ns = None

_NC_CACHE = {}


def _build_nc(loop_r=None, ko_limit=None):
    nc = bacc.Bacc()

    zT = nc.dram_tensor("zT", [K, B_L], BF16, kind="ExternalInput")
    zQ = nc.dram_tensor("zQ", [K, B_L], FP8, kind="ExternalInput")
    cT = nc.dram_tensor("cT", [O_L, B_L], BF16, kind="ExternalInput")
    # bf16 weight units, flat per-oc-contiguous: [oc, kp, u, p] with
    # u 0..15 = gate c ko 0..15, u 16..23 = gate o ko 8..15 (x 2^15)
    wB = nc.dram_tensor("wB", [OC, P, BU, P], BF16, kind="ExternalInput")
    # fp8 weight units: u 0..15 = f, 16..31 = i, 32..39 = o ko 0..7
    wQ = nc.dram_tensor("wQ", [OC, P, QU, P], FP8, kind="ExternalInput")
    # gate-fused biases: [p, oc, gate] in GATES order
    bA = nc.dram_tensor("bA", [P, OC, NG], F32, kind="ExternalInput")
    hT = nc.dram_tensor("hT", [O_L, B_L], BF16, kind="ExternalOutput")

    zT_t = zT[:, :].rearrange("(ko kp) b -> kp ko b", kp=P)    # [128,16,1024]
    zQ_t = zQ[:, :].rearrange("(ko kp) b -> kp ko b", kp=P)
    cT_t = cT[:, :].rearrange("(oc p) b -> p oc b", p=P)       # [128,4,1024]
    hT_t = hT[:, :].rearrange("(oc p) b -> p oc b", p=P)

    sig = mybir.ActivationFunctionType.Sigmoid
    tanh = mybir.ActivationFunctionType.Tanh
    ko_hi = ko_limit or KO
    assert ko_hi % 2 == 0, "fp8 DoubleRow needs an even ko count"

    with tile.TileContext(nc) as tc:
        with (
            tc.tile_pool(name="zpool", bufs=1) as zpool,
            tc.tile_pool(name="cpool", bufs=1) as cpool,
            tc.tile_pool(name="bpool", bufs=1) as bpool,
            tc.tile_pool(name="wpool", bufs=1) as wpool,
            tc.tile_pool(name="gates", bufs=1) as gpool,
            tc.tile_pool(name="temps", bufs=2) as tpool,
            tc.tile_pool(name="psum", bufs=8, space="PSUM") as psum_pool,
        ):
            # explicit static slots (bufs=1 pools, distinct tags)
            z_slots = [
                zpool.tile([P, KO, B_L], BF16, tag=f"z{s}", name=f"z{s}")
                for s in range(ZS)
            ]
            zq_slots = [
                zpool.tile([P, KO, B_L], FP8, tag=f"zq{s}", name=f"zq{s}")
                for s in range(ZS)
            ]
            wb_slots = [
                wpool.tile([P, BU, P], BF16, tag=f"wb{s}", name=f"wb{s}")
                for s in range(WS)
            ]
            wq_slots = [
                wpool.tile([P, QU, P], FP8, tag=f"wq{s}", name=f"wq{s}")
                for s in range(WS)
            ]
            c_slots = [
                cpool.tile([P, B_L], BF16, tag=f"c{s}", name=f"c{s}")
                for s in range(CS)
            ]
            b_sb = bpool.tile([P, OC, NG], F32, tag="b", name="b")

            def load_z(copy, quarter=None):
                # quarter loads flatten the DMA duty cycle (see docstring)
                if quarter is None:
                    nc.sync.dma_start(
                        z_slots[copy % ZS][:, :, :], zT_t[:, :, :]
                    )
                    nc.sync.dma_start(
                        zq_slots[copy % ZS][:, :, :], zQ_t[:, :, :]
                    )
                else:
                    ksl = slice(quarter * 4, (quarter + 1) * 4)
                    nc.sync.dma_start(
                        z_slots[copy % ZS][:, ksl, :], zT_t[:, ksl, :]
                    )
                    nc.sync.dma_start(
                        zq_slots[copy % ZS][:, ksl, :], zQ_t[:, ksl, :]
                    )

            def load_w(u):
                # weights for global unit u (copy u//OC, oc u%OC) -> slot u%WS
                nc.sync.dma_start(
                    wb_slots[u % WS][:, :, :], wB[u % OC, :, :, :]
                )
                nc.sync.dma_start(
                    wq_slots[u % WS][:, :, :], wQ[u % OC, :, :, :]
                )

            def load_c(u):
                nc.scalar.dma_start(c_slots[u % CS][:, :], cT_t[:, u % OC, :])

            def compute_unit(u):
                # ko-outer/gate-inner over one oc: the PSUM bank rotates on
                # every matmul so each matmul's drain overlaps the next
                # one's fill.  f/i ride fp8 DoubleRow on even ko rounds
                # (256 k per matmul), c/o ride bf16 every round.
                copy, oc = divmod(u, OC)
                z_sb = z_slots[copy % ZS]
                zq_sb = zq_slots[copy % ZS]
                wb_sb = wb_slots[u % WS]
                wq_sb = wq_slots[u % WS]
                c_sb = c_slots[u % CS]
                gate_sb = {}
                cf_sb = {}
                ps8 = {
                    (g, nb): psum_pool.tile([P, NT], F32, tag="ps", name="ps")
                    for g in GATES for nb in range(NB)
                }
                def mm_fp8(g, uoff, ko, start, stop):
                    for nb in range(NB):
                        nc.tensor.matmul(
                            ps8[(g, nb)][:, :],
                            lhsT=wq_sb[:, uoff + ko:uoff + ko + 2, :],
                            rhs=zq_sb[:, ko:ko + 2, nb * NT:(nb + 1) * NT],
                            start=start,
                            stop=stop,
                            perf_mode=mybir.MatmulPerfMode.DoubleRow,
                        )

                def mm_bf16(g, wu, ko, start, stop):
                    for nb in range(NB):
                        nc.tensor.matmul(
                            ps8[(g, nb)][:, :],
                            lhsT=wb_sb[:, wu, :],
                            rhs=z_sb[:, ko, nb * NT:(nb + 1) * NT],
                            start=start,
                            stop=stop,
                        )

                for ko in range(ko_hi):
                    if ko % 2 == 0:
                        mm_fp8("f", 0, ko, ko == 0, ko == ko_hi - 2)
                        mm_fp8("i", KO, ko, ko == 0, ko == ko_hi - 2)
                        if ko < MIX_O:
                            mm_fp8("o", 2 * KO, ko, ko == 0, False)
                    mm_bf16("c", ko, ko, ko == 0, ko == ko_hi - 1)
                    if ko >= MIX_O:
                        mm_bf16("o", KO + ko - MIX_O, ko,
                                False, ko == ko_hi - 1)
                for gi, g in enumerate(GATES):
                    scale = 1.0 if g == "c" else QSCALE
                    for nb in range(NB):
                        gt = gpool.tile(
                            [P, NT], F32, tag=f"gate_{g}_{nb}",
                            name=f"gate_{g}_{nb}",
                        )
                        nc.scalar.activation(
                            gt[:, :], ps8[(g, nb)][:, :],
                            tanh if g == "c" else sig,
                            bias=b_sb[:, oc, gi:gi + 1],
                            scale=scale,
                        )
                        gate_sb[(g, nb)] = gt
                    if g == "c":
                        # tanh(c*f + i*g) is independent of gate o — emit now
                        # so only mul+store remain after the last matmul
                        for nb in range(NB):
                            bsl = slice(nb * NT, (nb + 1) * NT)
                            cf = tpool.tile([P, NT], F32, tag="cf",
                                            name=f"cf_{nb}")
                            nc.vector.tensor_mul(
                                cf[:, :], c_sb[:, bsl],
                                gate_sb[("f", nb)][:, :],
                            )
                            ig = tpool.tile([P, NT], F32, tag="ig", name="ig")
                            nc.vector.tensor_mul(
                                ig[:, :], gate_sb[("i", nb)][:, :],
                                gate_sb[("c", nb)][:, :],
                            )
                            nc.vector.tensor_add(
                                cf[:, :], cf[:, :], ig[:, :]
                            )
                            nc.scalar.activation(cf[:, :], cf[:, :], tanh)
                            cf_sb[nb] = cf
                for nb in range(NB):
                    bsl = slice(nb * NT, (nb + 1) * NT)
                    ho = tpool.tile([P, NT], BF16, tag="ho", name="ho")
                    nc.vector.tensor_mul(
                        ho[:, :], cf_sb[nb][:, :], gate_sb[("o", nb)][:, :]
                    )
                    nc.scalar.dma_start(hT_t[:, oc, bsl], ho[:, :])

            # ---- prologue: first copy's working set -----------------------
            nc.scalar.dma_start(b_sb[:, :, :], bA[:, :, :])
            load_z(0)
            load_w(0)
            load_w(1)
            load_c(0)
            load_c(1)

            if loop_r:
                with tc.For_i(0, loop_r // NCOPIES, 1):
                    for u in range(NU):
                        # prefetch one quarter of copy c+1's z per unit —
                        # slot (c+1)%ZS was last read by copy c-1, already
                        # retired; all 4 quarters land across copy c
                        load_z(u // OC + 1, quarter=u % OC)
                        load_w(u + 2)          # slot (u+2)%WS, wraps to next
                        load_c(u + 2)
                        if u == 1:
                            nc.scalar.dma_start(b_sb[:, :, :], bA[:, :, :])
                        compute_unit(u)
            else:
                # one-shot: single copy, stream w/c two units ahead
                for u in range(OC):
                    if u + 2 < OC:
                        load_w(u + 2)
                        load_c(u + 2)
                    compute_unit(u)

    nc.finalize()
    return nc


def _get_nc():
    if "nc" not in _NC_CACHE:
        _NC_CACHE["nc"] = _build_nc()
    return _NC_CACHE["nc"]


def _shard_inputs(x, h, c, w_f, b_f, w_i, b_i, w_c, b_c, w_o, b_o):
    ws = {"f": w_f, "i": w_i, "c": w_c, "o": w_o}
    bz = {"f": b_f, "i": b_i, "c": b_c, "o": b_o}
    f32 = np.float32

    # per-out-group fused weight/bias shards (shared by the 4 batch groups)
    wB_sh = {}
    wQ_sh = {}
    bA_sh = {}
    for j in range(RO):
        cols = slice(j * O_L, (j + 1) * O_L)

        def units(g):
            # [KO, kp, OC, p] f32 view of w_g's column shard
            return (
                np.asarray(ws[g][:, cols], dtype=f32)
                .reshape(KO, P, OC, P)
            )

        # bf16 units: c ko 0..15, then o ko MIX_O..15 scaled by 2^15 so its
        # PSUM partial sits on the same scale as the fp8 o partial
        wb = np.concatenate(
            [units("c"), units("o")[MIX_O:] * (SZ * SW)], axis=0
        )  # [BU, kp, OC, p]
        wB_sh[j] = np.ascontiguousarray(
            wb.transpose(2, 1, 0, 3).astype(NP_BF16)
        )
        # fp8 units: f ko 0..15, i ko 0..15, o ko 0..MIX_O-1 (x SW)
        wq = np.concatenate(
            [units("f"), units("i"), units("o")[:MIX_O]], axis=0
        )  # [QU, kp, OC, p]
        wQ_sh[j] = np.ascontiguousarray(
            np.clip(wq.transpose(2, 1, 0, 3) * SW, -240.0, 240.0)
            .astype(NP_FP8)
        )
        bA_sh[j] = np.ascontiguousarray(
            np.stack(
                [np.asarray(bz[g], dtype=f32).reshape(-1)[cols]
                 .reshape(OC, P).T
                 for g in GATES],
                axis=2,
            )
        )

    in_maps = []
    for i in range(RB):
        rows = slice(i * B_L, (i + 1) * B_L)
        zTf = np.concatenate(
            [np.asarray(x[rows], dtype=f32), np.asarray(h[rows], dtype=f32)],
            axis=1,
        ).T  # [K, B_L] f32
        zT = np.ascontiguousarray(zTf.astype(NP_BF16))
        zQv = np.ascontiguousarray(
            np.clip(zTf * SZ, -240.0, 240.0).astype(NP_FP8)
        )
        for j in range(RO):
            cT = np.ascontiguousarray(
                np.asarray(c[rows, j * O_L:(j + 1) * O_L], dtype=f32).T
                .astype(NP_BF16)
            )
            in_maps.append(
                {"zT": zT, "zQ": zQv, "cT": cT,
                 "wB": wB_sh[j], "wQ": wQ_sh[j], "bA": bA_sh[j]}
            )
    return in_maps


def _run(in_maps, trace=False, trace_cores=None):
    global last_exec_time_ns
    nc = _get_nc()
    res = run_bass_kernel_spmd(
        nc, in_maps, list(range(RB * RO)),
        trace=trace, trace_cores=trace_cores,
    )
    if trace:
        last_exec_time_ns = res.exec_time_ns
    return res.results


def kernel(x, h, c, w_f, b_f, w_i, b_i, w_c, b_c, w_o, b_o):
    in_maps = _shard_inputs(
        x, h, c, w_f, b_f, w_i, b_i, w_c, b_c, w_o, b_o
    )
    results = _run(in_maps)
    out = np.empty((B_FULL, OUT), np.float32)
    for i in range(RB):
        for j in range(RO):
            shard = results[i * RO + j]["hT"]  # [O_L, B_L] bf16
            out[i * B_L:(i + 1) * B_L, j * O_L:(j + 1) * O_L] = (
                shard.astype(np.float32).T
            )
    return out


# revision 18
# speedup vs baseline: 1.5489x; 1.0070x over previous
"""LSTM layer kernel for Trainium2 (8 NeuronCores, Bass/Tile).

Reference computation (fp32):
    z = concat([x, h], axis=1)                 # [B, IN+OUT]
    f = sigmoid(z @ w_f + b_f)
    i = sigmoid(z @ w_i + b_i)
    g = tanh   (z @ w_c + b_c)
    o = sigmoid(z @ w_o + b_o)
    c_new = c * f + i * g
    h_new = tanh(c_new) * o                    # [B, OUT]

Shapes: B=4096, IN=OUT=1024, K=IN+OUT=2048.
Sharding (8 cores): 4 batch-groups x 2 output-column-groups; core (i, j)
computes h_new[i*1024:(i+1)*1024, j*512:(j+1)*512].  No collectives.

The PE is the sole bottleneck (ACT/DVE sit at ~13% duty), so the win over
the all-bf16 schedule is cutting PE cycles with fp8 DoubleRow matmuls:
e4m3 operands + perf_mode=DoubleRow contract 256 k-elements per 512-cycle
matmul (2 fp8 products per PE cell per cycle) — 2x bf16 FLOP rate.  Pure
e4m3 for all four gates fails the 2e-2 gate (measured 3.1e-2 end to end:
~2.4% RMS quantization noise per operand -> 3.8e-2 pre-activation noise),
but the gates' error sensitivities are skewed (candidate tanh 2.4e-2,
output 1.5e-2, forget 1.1e-2, input 0.8e-2 when quantized alone).
Allocation chosen from a numpy sim that tracks hw to ~4 digits (verified
twice on device): forget/input gates fully fp8 (8 DoubleRow matmuls
each), candidate stays bf16 (16), output runs its first MIX_O=12 ko
chunks fp8 + last 4 bf16 (6 DoubleRow + 4 bf16 matmuls): 42/64 of the
bf16 slot count, rel err 1.864e-2 on hw (seed-to-seed spread measured
<0.01% — the margin is deterministic, not statistical; MIX_O=16 would be
2.00e-2, right at the gate).

fp8 scales are fixed powers of two (z*32, w*1024; e4m3 relative error is
scale-invariant — a sweep moved rel err <0.2% — only clipping matters
and |z|<7.5sigma, |w|<2.3x absmax stay in +-240), undone by the
activation's scale operand (2^-15) at zero cost.  The o-gate's bf16
weight chunks are pre-scaled by 2^15 (exact exponent shift) so its bf16
PSUM partial lands on the same scale as its fp8 partial.  Quantization
runs host-side from the f32 inputs (same class of prep as the baseline's
bf16 casts/transposes).

Device layout: contraction dim (k) and output-channel dim (o) sit on SBUF
partitions; zT [2048,1024] bf16 + zQ e4m3 are the moving operands, the
gate-fused weights (wB bf16 for c/o, wQ e4m3 for f/i) are stationary,
out = w.T @ zT lands in PSUM as [o, b] so bias/sigmoid/tanh run straight
out of PSUM.  wQ is laid out gate-outer ([kp, g, ko, p]) so the DoubleRow
lhsT slice [:, g, ko:ko+2, :] has the pair dim contiguous — the exact AP
shape validated on hw.  c rides bf16 (error contribution ~3e-3 rel,
negligible vs the fp8 noise) to keep SBUF at ~179KB/partition.

Scheduling (inherited from the bf16 baseline, measured there):
  - Tile's For_i puts an all-engine barrier on every back edge; the body
    holds NCOPIES=8 problem copies = 32 "oc units" so the ~2.2us barrier
    amortizes /8, and DMAs issued late in iteration n prefetch what n+1
    starts with (slots persist across the barrier).
  - Unit u's compute is preceded by the DMA for unit u+2's weights/c and
    one quarter of the next copy's z (both dtypes), flattening the DMA
    duty cycle (co-running DMA measurably slows the matmul stream).
  - Within a unit the matmul order is ko-outer/gate-inner: the PSUM bank
    rotates on every matmul (all 8 banks accumulate concurrently),
    overlapping each matmul's drain with the next one's fill.  The f/i
    DoubleRow matmuls ride the even ko rounds and stop at ko=14, so
    their ACT drains overlap the last c/o matmul rounds.
  - z/w loads ride the sync HWDGE ring; c/bias loads and h stores ride
    the scalar ring, so a store waiting on compute never head-of-line
    blocks a load.
  - Slot cycling is static: z/zQ 2 slots (copy parity), w 4 slots, c 4
    slots; unit u reads slot u%4 and the prefetch writes slot (u+2)%4,
    consistent across the loop wrap since 32%4==0.
"""

import numpy as np
import ml_dtypes

import concourse.bass as bass
import concourse.tile as tile
from concourse import bacc
from concourse import mybir
from concourse.bass_utils import run_bass_kernel_spmd

P = 128
B_FULL, IN, OUT = 4096, 1024, 1024
K = IN + OUT                 # 2048 contraction
RB, RO = 4, 2                # batch-shards x out-col-shards = 8 cores
B_L = B_FULL // RB           # 1024 batch rows per core
O_L = OUT // RO              # 512 out cols per core
KO = K // P                  # 16 k-chunks
OC = O_L // P                # 4 out chunks per core
NG = 4                       # gates
NT = 512                     # moving free dim per matmul (one PSUM bank)
NB = B_L // NT               # 2 batch tiles
NCOPIES = 8                  # kernel copies per hardware-loop iteration
ZS = 2                       # z slots (copy parity)
NU = NCOPIES * OC            # oc-units per body
WS = 4                       # w slots (divides NU)
CS = 4                       # c slots (divides NU)

SZ = 32.0                    # fp8 scale for z (power of 2: exact)
SW = 1024.0                  # fp8 scale for w_f/w_i/w_o
QSCALE = 1.0 / (SZ * SW)     # 2^-15, folded into the activation
MIX_O = 12                   # o-gate ko chunks in fp8 (rest bf16)
QU = 2 * KO + MIX_O          # fp8 weight units: f 16, i 16, o 8
BU = KO + (KO - MIX_O)       # bf16 weight units: c 16, o 8

F32 = mybir.dt.float32
BF16 = mybir.dt.bfloat16
FP8 = mybir.dt.float8e4
NP_BF16 = ml_dtypes.bfloat16
NP_FP8 = ml_dtypes.float8_e4m3   # TRN FP8_EXP4 bit pattern (max +-240)
GATES = ("f", "i", "c", "o")
FP8_GATES = ("f", "i")       # low-sensitivity gates -> e4m3 DoubleRow
BF_GATES = ("c", "o")        # high-sensitivity gates -> bf16

last_exec_time_ns = None

_NC_CACHE = {}


def _build_nc(loop_r=None, ko_limit=None, extra_dma=0, w_rings=None):
    """extra_dma: dummy z-quarter loads per unit (DMA co-run probe).
    w_rings: (wb_engine, wq_engine) names, default ('sync', 'sync')."""
    nc = bacc.Bacc()

    zT = nc.dram_tensor("zT", [K, B_L], BF16, kind="ExternalInput")
    zQ = nc.dram_tensor("zQ", [K, B_L], FP8, kind="ExternalInput")
    cT = nc.dram_tensor("cT", [O_L, B_L], BF16, kind="ExternalInput")
    # bf16 weight units, flat per-oc-contiguous: [oc, kp, u, p] with
    # u 0..15 = gate c ko 0..15, u 16..23 = gate o ko 8..15 (x 2^15)
    wB = nc.dram_tensor("wB", [OC, P, BU, P], BF16, kind="ExternalInput")
    # fp8 weight units: u 0..15 = f, 16..31 = i, 32..39 = o ko 0..7
    wQ = nc.dram_tensor("wQ", [OC, P, QU, P], FP8, kind="ExternalInput")
    # gate-fused biases: [p, oc, gate] in GATES order
    bA = nc.dram_tensor("bA", [P, OC, NG], F32, kind="ExternalInput")
    hT = nc.dram_tensor("hT", [O_L, B_L], BF16, kind="ExternalOutput")

    zT_t = zT[:, :].rearrange("(ko kp) b -> kp ko b", kp=P)    # [128,16,1024]
    zQ_t = zQ[:, :].rearrange("(ko kp) b -> kp ko b", kp=P)
    cT_t = cT[:, :].rearrange("(oc p) b -> p oc b", p=P)       # [128,4,1024]
    hT_t = hT[:, :].rearrange("(oc p) b -> p oc b", p=P)

    sig = mybir.ActivationFunctionType.Sigmoid
    tanh = mybir.ActivationFunctionType.Tanh
    ko_hi = ko_limit or KO
    assert ko_hi % 2 == 0, "fp8 DoubleRow needs an even ko count"

    with tile.TileContext(nc) as tc:
        with (
            tc.tile_pool(name="zpool", bufs=1) as zpool,
            tc.tile_pool(name="cpool", bufs=1) as cpool,
            tc.tile_pool(name="bpool", bufs=1) as bpool,
            tc.tile_pool(name="wpool", bufs=1) as wpool,
            tc.tile_pool(name="gates", bufs=1) as gpool,
            tc.tile_pool(name="temps", bufs=2) as tpool,
            tc.tile_pool(name="psum", bufs=8, space="PSUM") as psum_pool,
        ):
            # explicit static slots (bufs=1 pools, distinct tags)
            z_slots = [
                zpool.tile([P, KO, B_L], BF16, tag=f"z{s}", name=f"z{s}")
                for s in range(ZS)
            ]
            zq_slots = [
                zpool.tile([P, KO, B_L], FP8, tag=f"zq{s}", name=f"zq{s}")
                for s in range(ZS)
            ]
            wb_slots = [
                wpool.tile([P, BU, P], BF16, tag=f"wb{s}", name=f"wb{s}")
                for s in range(WS)
            ]
            wq_slots = [
                wpool.tile([P, QU, P], FP8, tag=f"wq{s}", name=f"wq{s}")
                for s in range(WS)
            ]
            c_slots = [
                cpool.tile([P, B_L], BF16, tag=f"c{s}", name=f"c{s}")
                for s in range(CS)
            ]
            b_sb = bpool.tile([P, OC, NG], F32, tag="b", name="b")

            def load_z(copy, quarter=None):
                # quarter loads flatten the DMA duty cycle (see docstring)
                if quarter is None:
                    nc.sync.dma_start(
                        z_slots[copy % ZS][:, :, :], zT_t[:, :, :]
                    )
                    nc.sync.dma_start(
                        zq_slots[copy % ZS][:, :, :], zQ_t[:, :, :]
                    )
                else:
                    ksl = slice(quarter * 4, (quarter + 1) * 4)
                    nc.sync.dma_start(
                        z_slots[copy % ZS][:, ksl, :], zT_t[:, ksl, :]
                    )
                    nc.sync.dma_start(
                        zq_slots[copy % ZS][:, ksl, :], zQ_t[:, ksl, :]
                    )

            wb_eng = getattr(nc, (w_rings or ("sync", "sync"))[0])
            wq_eng = getattr(nc, (w_rings or ("sync", "sync"))[1])

            def load_w(u):
                # weights for global unit u (copy u//OC, oc u%OC) -> slot u%WS
                wb_eng.dma_start(
                    wb_slots[u % WS][:, :, :], wB[u % OC, :, :, :]
                )
                wq_eng.dma_start(
                    wq_slots[u % WS][:, :, :], wQ[u % OC, :, :, :]
                )

            def load_c(u):
                nc.scalar.dma_start(c_slots[u % CS][:, :], cT_t[:, u % OC, :])

            def compute_unit(u):
                # ko-outer/gate-inner over one oc: the PSUM bank rotates on
                # every matmul so each matmul's drain overlaps the next
                # one's fill.  f/i ride fp8 DoubleRow on even ko rounds
                # (256 k per matmul), c/o ride bf16 every round.
                copy, oc = divmod(u, OC)
                z_sb = z_slots[copy % ZS]
                zq_sb = zq_slots[copy % ZS]
                wb_sb = wb_slots[u % WS]
                wq_sb = wq_slots[u % WS]
                c_sb = c_slots[u % CS]
                gate_sb = {}
                cf_sb = {}
                ps8 = {
                    (g, nb): psum_pool.tile([P, NT], F32, tag="ps", name="ps")
                    for g in GATES for nb in range(NB)
                }
                def mm_fp8(g, uoff, ko, start, stop):
                    for nb in range(NB):
                        nc.tensor.matmul(
                            ps8[(g, nb)][:, :],
                            lhsT=wq_sb[:, uoff + ko:uoff + ko + 2, :],
                            rhs=zq_sb[:, ko:ko + 2, nb * NT:(nb + 1) * NT],
                            start=start,
                            stop=stop,
                            perf_mode=mybir.MatmulPerfMode.DoubleRow,
                        )

                def mm_bf16(g, wu, ko, start, stop):
                    for nb in range(NB):
                        nc.tensor.matmul(
                            ps8[(g, nb)][:, :],
                            lhsT=wb_sb[:, wu, :],
                            rhs=z_sb[:, ko, nb * NT:(nb + 1) * NT],
                            start=start,
                            stop=stop,
                        )

                for ko in range(ko_hi):
                    if ko % 2 == 0:
                        mm_fp8("f", 0, ko, ko == 0, ko == ko_hi - 2)
                        mm_fp8("i", KO, ko, ko == 0, ko == ko_hi - 2)
                        if ko < MIX_O:
                            mm_fp8("o", 2 * KO, ko, ko == 0, False)
                    mm_bf16("c", ko, ko, ko == 0, ko == ko_hi - 1)
                    if ko >= MIX_O:
                        mm_bf16("o", KO + ko - MIX_O, ko,
                                False, ko == ko_hi - 1)
                for gi, g in enumerate(GATES):
                    scale = 1.0 if g == "c" else QSCALE
                    for nb in range(NB):
                        gt = gpool.tile(
                            [P, NT], F32, tag=f"gate_{g}_{nb}",
                            name=f"gate_{g}_{nb}",
                        )
                        nc.scalar.activation(
                            gt[:, :], ps8[(g, nb)][:, :],
                            tanh if g == "c" else sig,
                            bias=b_sb[:, oc, gi:gi + 1],
                            scale=scale,
                        )
                        gate_sb[(g, nb)] = gt
                    if g == "c":
                        # tanh(c*f + i*g) is independent of gate o — emit now
                        # so only mul+store remain after the last matmul
                        for nb in range(NB):
                            bsl = slice(nb * NT, (nb + 1) * NT)
                            cf = tpool.tile([P, NT], F32, tag="cf",
                                            name=f"cf_{nb}")
                            nc.vector.tensor_mul(
                                cf[:, :], c_sb[:, bsl],
                                gate_sb[("f", nb)][:, :],
                            )
                            ig = tpool.tile([P, NT], F32, tag="ig", name="ig")
                            nc.vector.tensor_mul(
                                ig[:, :], gate_sb[("i", nb)][:, :],
                                gate_sb[("c", nb)][:, :],
                            )
                            nc.vector.tensor_add(
                                cf[:, :], cf[:, :], ig[:, :]
                            )
                            nc.scalar.activation(cf[:, :], cf[:, :], tanh)
                            cf_sb[nb] = cf
                for nb in range(NB):
                    bsl = slice(nb * NT, (nb + 1) * NT)
                    ho = tpool.tile([P, NT], BF16, tag="ho", name="ho")
                    nc.vector.tensor_mul(
                        ho[:, :], cf_sb[nb][:, :], gate_sb[("o", nb)][:, :]
                    )
                    nc.scalar.dma_start(hT_t[:, oc, bsl], ho[:, :])

            # ---- prologue: first copy's working set -----------------------
            nc.scalar.dma_start(b_sb[:, :, :], bA[:, :, :])
            load_z(0)
            load_w(0)
            load_w(1)
            load_c(0)
            load_c(1)

            scratch = (
                zpool.tile([P, 4, B_L], BF16, tag="scr", name="scr")
                if extra_dma else None
            )

            if loop_r:
                with tc.For_i(0, loop_r // NCOPIES, 1):
                    for u in range(NU):
                        # prefetch one quarter of copy c+1's z per unit —
                        # slot (c+1)%ZS was last read by copy c-1, already
                        # retired; all 4 quarters land across copy c
                        load_z(u // OC + 1, quarter=u % OC)
                        load_w(u + 2)          # slot (u+2)%WS, wraps to next
                        load_c(u + 2)
                        for _ in range(extra_dma):
                            ksl = slice((u % OC) * 4, (u % OC) * 4 + 4)
                            nc.sync.dma_start(
                                scratch[:, :, :], zT_t[:, ksl, :]
                            )
                        if u == 1:
                            nc.scalar.dma_start(b_sb[:, :, :], bA[:, :, :])
                        compute_unit(u)
            else:
                # one-shot: single copy, stream w/c two units ahead
                for u in range(OC):
                    if u + 2 < OC:
                        load_w(u + 2)
                        load_c(u + 2)
                    compute_unit(u)

    nc.finalize()
    return nc


def _get_nc():
    if "nc" not in _NC_CACHE:
        _NC_CACHE["nc"] = _build_nc()
    return _NC_CACHE["nc"]


def _shard_inputs(x, h, c, w_f, b_f, w_i, b_i, w_c, b_c, w_o, b_o):
    ws = {"f": w_f, "i": w_i, "c": w_c, "o": w_o}
    bz = {"f": b_f, "i": b_i, "c": b_c, "o": b_o}
    f32 = np.float32

    # per-out-group fused weight/bias shards (shared by the 4 batch groups)
    wB_sh = {}
    wQ_sh = {}
    bA_sh = {}
    for j in range(RO):
        cols = slice(j * O_L, (j + 1) * O_L)

        def units(g):
            # [KO, kp, OC, p] f32 view of w_g's column shard
            return (
                np.asarray(ws[g][:, cols], dtype=f32)
                .reshape(KO, P, OC, P)
            )

        # bf16 units: c ko 0..15, then o ko MIX_O..15 scaled by 2^15 so its
        # PSUM partial sits on the same scale as the fp8 o partial
        wb = np.concatenate(
            [units("c"), units("o")[MIX_O:] * (SZ * SW)], axis=0
        )  # [BU, kp, OC, p]
        wB_sh[j] = np.ascontiguousarray(
            wb.transpose(2, 1, 0, 3).astype(NP_BF16)
        )
        # fp8 units: f ko 0..15, i ko 0..15, o ko 0..MIX_O-1 (x SW)
        wq = np.concatenate(
            [units("f"), units("i"), units("o")[:MIX_O]], axis=0
        )  # [QU, kp, OC, p]
        wQ_sh[j] = np.ascontiguousarray(
            np.clip(wq.transpose(2, 1, 0, 3) * SW, -240.0, 240.0)
            .astype(NP_FP8)
        )
        bA_sh[j] = np.ascontiguousarray(
            np.stack(
                [np.asarray(bz[g], dtype=f32).reshape(-1)[cols]
                 .reshape(OC, P).T
                 for g in GATES],
                axis=2,
            )
        )

    in_maps = []
    for i in range(RB):
        rows = slice(i * B_L, (i + 1) * B_L)
        zTf = np.concatenate(
            [np.asarray(x[rows], dtype=f32), np.asarray(h[rows], dtype=f32)],
            axis=1,
        ).T  # [K, B_L] f32
        zT = np.ascontiguousarray(zTf.astype(NP_BF16))
        zQv = np.ascontiguousarray(
            np.clip(zTf * SZ, -240.0, 240.0).astype(NP_FP8)
        )
        for j in range(RO):
            cT = np.ascontiguousarray(
                np.asarray(c[rows, j * O_L:(j + 1) * O_L], dtype=f32).T
                .astype(NP_BF16)
            )
            in_maps.append(
                {"zT": zT, "zQ": zQv, "cT": cT,
                 "wB": wB_sh[j], "wQ": wQ_sh[j], "bA": bA_sh[j]}
            )
    return in_maps


def _run(in_maps, trace=False, trace_cores=None):
    global last_exec_time_ns
    nc = _get_nc()
    res = run_bass_kernel_spmd(
        nc, in_maps, list(range(RB * RO)),
        trace=trace, trace_cores=trace_cores,
    )
    if trace:
        last_exec_time_ns = res.exec_time_ns
    return res.results


def kernel(x, h, c, w_f, b_f, w_i, b_i, w_c, b_c, w_o, b_o):
    in_maps = _shard_inputs(
        x, h, c, w_f, b_f, w_i, b_i, w_c, b_c, w_o, b_o
    )
    results = _run(in_maps)
    out = np.empty((B_FULL, OUT), np.float32)
    for i in range(RB):
        for j in range(RO):
            shard = results[i * RO + j]["hT"]  # [O_L, B_L] bf16
            out[i * B_L:(i + 1) * B_L, j * O_L:(j + 1) * O_L] = (
                shard.astype(np.float32).T
            )
    return out


# revision 21
# speedup vs baseline: 1.5666x; 1.0114x over previous
"""LSTM layer kernel for Trainium2 (8 NeuronCores, Bass/Tile).

Reference computation (fp32):
    z = concat([x, h], axis=1)                 # [B, IN+OUT]
    f = sigmoid(z @ w_f + b_f)
    i = sigmoid(z @ w_i + b_i)
    g = tanh   (z @ w_c + b_c)
    o = sigmoid(z @ w_o + b_o)
    c_new = c * f + i * g
    h_new = tanh(c_new) * o                    # [B, OUT]

Shapes: B=4096, IN=OUT=1024, K=IN+OUT=2048.
Sharding (8 cores): 4 batch-groups x 2 output-column-groups; core (i, j)
computes h_new[i*1024:(i+1)*1024, j*512:(j+1)*512].  No collectives.

The PE is the sole bottleneck (ACT/DVE sit at ~13% duty), so the win over
the all-bf16 schedule is cutting PE cycles with fp8 DoubleRow matmuls:
e4m3 operands + perf_mode=DoubleRow contract 256 k-elements per 512-cycle
matmul (2 fp8 products per PE cell per cycle) — 2x bf16 FLOP rate.  Pure
e4m3 for all four gates fails the 2e-2 gate (measured 3.1e-2 end to end:
~2.4% RMS quantization noise per operand -> 3.8e-2 pre-activation noise),
but the gates' error sensitivities are skewed (candidate tanh 2.4e-2,
output 1.5e-2, forget 1.1e-2, input 0.8e-2 when quantized alone).
Allocation chosen from a numpy sim that tracks hw to ~4 digits (verified
twice on device): forget/input gates fully fp8 (8 DoubleRow matmuls
each), candidate stays bf16 (16), output runs its first MIX_O=12 ko
chunks fp8 + last 4 bf16 (6 DoubleRow + 4 bf16 matmuls): 42/64 of the
bf16 slot count, rel err 1.864e-2 on hw (seed-to-seed spread measured
<0.01% — the margin is deterministic, not statistical; MIX_O=16 would be
2.00e-2, right at the gate).

fp8 scales are fixed powers of two (z*32, w*1024; e4m3 relative error is
scale-invariant — a sweep moved rel err <0.2% — only clipping matters
and |z|<7.5sigma, |w|<2.3x absmax stay in +-240), undone by the
activation's scale operand (2^-15) at zero cost.  The o-gate's bf16
weight chunks are pre-scaled by 2^15 (exact exponent shift) so its bf16
PSUM partial lands on the same scale as its fp8 partial.  Quantization
runs host-side from the f32 inputs (same class of prep as the baseline's
bf16 casts/transposes).

Device layout: contraction dim (k) and output-channel dim (o) sit on SBUF
partitions; zT [2048,1024] bf16 + zQ e4m3 are the moving operands, the
gate-fused weights (wB bf16 for c/o, wQ e4m3 for f/i) are stationary,
out = w.T @ zT lands in PSUM as [o, b] so bias/sigmoid/tanh run straight
out of PSUM.  wQ is laid out gate-outer ([kp, g, ko, p]) so the DoubleRow
lhsT slice [:, g, ko:ko+2, :] has the pair dim contiguous — the exact AP
shape validated on hw.  c rides bf16 (error contribution ~3e-3 rel,
negligible vs the fp8 noise) to keep SBUF at ~179KB/partition.

Scheduling (inherited from the bf16 baseline, measured there):
  - Tile's For_i puts an all-engine barrier on every back edge; the body
    holds NCOPIES=8 problem copies = 32 "oc units" so the ~2.2us barrier
    amortizes /8, and DMAs issued late in iteration n prefetch what n+1
    starts with (slots persist across the barrier).
  - Unit u's compute is preceded by the DMA for unit u+2's weights/c and
    one quarter of the next copy's z (both dtypes), flattening the DMA
    duty cycle (co-running DMA measurably slows the matmul stream).
  - Within a unit the matmul order is ko-outer/gate-inner: the PSUM bank
    rotates on every matmul (all 8 banks accumulate concurrently),
    overlapping each matmul's drain with the next one's fill.  The f/i
    DoubleRow matmuls ride the even ko rounds and stop at ko=14, so
    their ACT drains overlap the last c/o matmul rounds.
  - z/w loads ride the sync HWDGE ring; c/bias loads and h stores ride
    the scalar ring, so a store waiting on compute never head-of-line
    blocks a load.
  - Slot cycling is static: z/zQ 2 slots (copy parity), w 4 slots, c 4
    slots; unit u reads slot u%4 and the prefetch writes slot (u+2)%4,
    consistent across the loop wrap since 32%4==0.
"""

import numpy as np
import ml_dtypes

import concourse.bass as bass
import concourse.tile as tile
from concourse import bacc
from concourse import mybir
from concourse.bass_utils import run_bass_kernel_spmd

P = 128
B_FULL, IN, OUT = 4096, 1024, 1024
K = IN + OUT                 # 2048 contraction
RB, RO = 4, 2                # batch-shards x out-col-shards = 8 cores
B_L = B_FULL // RB           # 1024 batch rows per core
O_L = OUT // RO              # 512 out cols per core
KO = K // P                  # 16 k-chunks
OC = O_L // P                # 4 out chunks per core
NG = 4                       # gates
NT = 512                     # moving free dim per matmul (one PSUM bank)
NB = B_L // NT               # 2 batch tiles
NCOPIES = 8                  # kernel copies per hardware-loop iteration
ZS = 2                       # z slots (copy parity)
NU = NCOPIES * OC            # oc-units per body
WS = 4                       # w slots (divides NU)
CS = 4                       # c slots (divides NU)

SZ = 32.0                    # fp8 scale for z (power of 2: exact)
SW = 1024.0                  # fp8 scale for w_f/w_i/w_o
QSCALE = 1.0 / (SZ * SW)     # 2^-15, folded into the activation
MIX_O = 14                   # o-gate ko chunks in fp8 (rest bf16)
QU = 2 * KO + MIX_O          # fp8 weight units: f 16, i 16, o 8
BU = KO + (KO - MIX_O)       # bf16 weight units: c 16, o 8

F32 = mybir.dt.float32
BF16 = mybir.dt.bfloat16
FP8 = mybir.dt.float8e4
NP_BF16 = ml_dtypes.bfloat16
NP_FP8 = ml_dtypes.float8_e4m3   # TRN FP8_EXP4 bit pattern (max +-240)
GATES = ("f", "i", "c", "o")
FP8_GATES = ("f", "i")       # low-sensitivity gates -> e4m3 DoubleRow
BF_GATES = ("c", "o")        # high-sensitivity gates -> bf16

last_exec_time_ns = None

_NC_CACHE = {}


def _build_nc(loop_r=None, ko_limit=None, extra_dma=0, w_rings=None,
              whole_z=False):
    """extra_dma: dummy z-quarter loads per unit (DMA co-run probe).
    w_rings: (wb_engine, wq_engine) names, default ('sync', 'sync').
    whole_z: prefetch next copy's z in one burst instead of quarters."""
    nc = bacc.Bacc()

    zT = nc.dram_tensor("zT", [K, B_L], BF16, kind="ExternalInput")
    zQ = nc.dram_tensor("zQ", [K, B_L], FP8, kind="ExternalInput")
    cT = nc.dram_tensor("cT", [O_L, B_L], BF16, kind="ExternalInput")
    # bf16 weight units, flat per-oc-contiguous: [oc, kp, u, p] with
    # u 0..15 = gate c ko 0..15, u 16..23 = gate o ko 8..15 (x 2^15)
    wB = nc.dram_tensor("wB", [OC, P, BU, P], BF16, kind="ExternalInput")
    # fp8 weight units: u 0..15 = f, 16..31 = i, 32..39 = o ko 0..7
    wQ = nc.dram_tensor("wQ", [OC, P, QU, P], FP8, kind="ExternalInput")
    # gate-fused biases: [p, oc, gate] in GATES order
    bA = nc.dram_tensor("bA", [P, OC, NG], F32, kind="ExternalInput")
    hT = nc.dram_tensor("hT", [O_L, B_L], BF16, kind="ExternalOutput")

    zT_t = zT[:, :].rearrange("(ko kp) b -> kp ko b", kp=P)    # [128,16,1024]
    zQ_t = zQ[:, :].rearrange("(ko kp) b -> kp ko b", kp=P)
    cT_t = cT[:, :].rearrange("(oc p) b -> p oc b", p=P)       # [128,4,1024]
    hT_t = hT[:, :].rearrange("(oc p) b -> p oc b", p=P)

    sig = mybir.ActivationFunctionType.Sigmoid
    tanh = mybir.ActivationFunctionType.Tanh
    ko_hi = ko_limit or KO
    assert ko_hi % 2 == 0, "fp8 DoubleRow needs an even ko count"

    with tile.TileContext(nc) as tc:
        with (
            tc.tile_pool(name="zpool", bufs=1) as zpool,
            tc.tile_pool(name="cpool", bufs=1) as cpool,
            tc.tile_pool(name="bpool", bufs=1) as bpool,
            tc.tile_pool(name="wpool", bufs=1) as wpool,
            tc.tile_pool(name="gates", bufs=1) as gpool,
            tc.tile_pool(name="temps", bufs=2) as tpool,
            tc.tile_pool(name="psum", bufs=8, space="PSUM") as psum_pool,
        ):
            # explicit static slots (bufs=1 pools, distinct tags)
            z_slots = [
                zpool.tile([P, KO, B_L], BF16, tag=f"z{s}", name=f"z{s}")
                for s in range(ZS)
            ]
            zq_slots = [
                zpool.tile([P, KO, B_L], FP8, tag=f"zq{s}", name=f"zq{s}")
                for s in range(ZS)
            ]
            wb_slots = [
                wpool.tile([P, BU, P], BF16, tag=f"wb{s}", name=f"wb{s}")
                for s in range(WS)
            ]
            wq_slots = [
                wpool.tile([P, QU, P], FP8, tag=f"wq{s}", name=f"wq{s}")
                for s in range(WS)
            ]
            c_slots = [
                cpool.tile([P, B_L], BF16, tag=f"c{s}", name=f"c{s}")
                for s in range(CS)
            ]
            b_sb = bpool.tile([P, OC, NG], F32, tag="b", name="b")

            def load_z(copy, quarter=None):
                # quarter loads flatten the DMA duty cycle (see docstring)
                if quarter is None:
                    nc.sync.dma_start(
                        z_slots[copy % ZS][:, :, :], zT_t[:, :, :]
                    )
                    nc.sync.dma_start(
                        zq_slots[copy % ZS][:, :, :], zQ_t[:, :, :]
                    )
                else:
                    ksl = slice(quarter * 4, (quarter + 1) * 4)
                    nc.sync.dma_start(
                        z_slots[copy % ZS][:, ksl, :], zT_t[:, ksl, :]
                    )
                    nc.sync.dma_start(
                        zq_slots[copy % ZS][:, ksl, :], zQ_t[:, ksl, :]
                    )

            wb_eng = getattr(nc, (w_rings or ("sync", "sync"))[0])
            wq_eng = getattr(nc, (w_rings or ("sync", "sync"))[1])

            def load_w(u):
                # weights for global unit u (copy u//OC, oc u%OC) -> slot u%WS
                wb_eng.dma_start(
                    wb_slots[u % WS][:, :, :], wB[u % OC, :, :, :]
                )
                wq_eng.dma_start(
                    wq_slots[u % WS][:, :, :], wQ[u % OC, :, :, :]
                )

            def load_c(u):
                nc.scalar.dma_start(c_slots[u % CS][:, :], cT_t[:, u % OC, :])

            def compute_unit(u):
                # ko-outer/gate-inner over one oc: the PSUM bank rotates on
                # every matmul so each matmul's drain overlaps the next
                # one's fill.  f/i ride fp8 DoubleRow on even ko rounds
                # (256 k per matmul), c/o ride bf16 every round.
                copy, oc = divmod(u, OC)
                z_sb = z_slots[copy % ZS]
                zq_sb = zq_slots[copy % ZS]
                wb_sb = wb_slots[u % WS]
                wq_sb = wq_slots[u % WS]
                c_sb = c_slots[u % CS]
                gate_sb = {}
                cf_sb = {}
                ps8 = {
                    (g, nb): psum_pool.tile([P, NT], F32, tag="ps", name="ps")
                    for g in GATES for nb in range(NB)
                }
                def mm_fp8(g, uoff, ko, start, stop):
                    for nb in range(NB):
                        nc.tensor.matmul(
                            ps8[(g, nb)][:, :],
                            lhsT=wq_sb[:, uoff + ko:uoff + ko + 2, :],
                            rhs=zq_sb[:, ko:ko + 2, nb * NT:(nb + 1) * NT],
                            start=start,
                            stop=stop,
                            perf_mode=mybir.MatmulPerfMode.DoubleRow,
                        )

                def mm_bf16(g, wu, ko, start, stop):
                    for nb in range(NB):
                        nc.tensor.matmul(
                            ps8[(g, nb)][:, :],
                            lhsT=wb_sb[:, wu, :],
                            rhs=z_sb[:, ko, nb * NT:(nb + 1) * NT],
                            start=start,
                            stop=stop,
                        )

                for ko in range(ko_hi):
                    if ko % 2 == 0:
                        mm_fp8("f", 0, ko, ko == 0, ko == ko_hi - 2)
                        mm_fp8("i", KO, ko, ko == 0, ko == ko_hi - 2)
                        if ko < MIX_O:
                            mm_fp8("o", 2 * KO, ko, ko == 0, False)
                    mm_bf16("c", ko, ko, ko == 0, ko == ko_hi - 1)
                    if ko >= MIX_O:
                        mm_bf16("o", KO + ko - MIX_O, ko,
                                False, ko == ko_hi - 1)
                for gi, g in enumerate(GATES):
                    scale = 1.0 if g == "c" else QSCALE
                    for nb in range(NB):
                        gt = gpool.tile(
                            [P, NT], F32, tag=f"gate_{g}_{nb}",
                            name=f"gate_{g}_{nb}",
                        )
                        nc.scalar.activation(
                            gt[:, :], ps8[(g, nb)][:, :],
                            tanh if g == "c" else sig,
                            bias=b_sb[:, oc, gi:gi + 1],
                            scale=scale,
                        )
                        gate_sb[(g, nb)] = gt
                    if g == "c":
                        # tanh(c*f + i*g) is independent of gate o — emit now
                        # so only mul+store remain after the last matmul
                        for nb in range(NB):
                            bsl = slice(nb * NT, (nb + 1) * NT)
                            cf = tpool.tile([P, NT], F32, tag="cf",
                                            name=f"cf_{nb}")
                            nc.vector.tensor_mul(
                                cf[:, :], c_sb[:, bsl],
                                gate_sb[("f", nb)][:, :],
                            )
                            ig = tpool.tile([P, NT], F32, tag="ig", name="ig")
                            nc.vector.tensor_mul(
                                ig[:, :], gate_sb[("i", nb)][:, :],
                                gate_sb[("c", nb)][:, :],
                            )
                            nc.vector.tensor_add(
                                cf[:, :], cf[:, :], ig[:, :]
                            )
                            nc.scalar.activation(cf[:, :], cf[:, :], tanh)
                            cf_sb[nb] = cf
                for nb in range(NB):
                    bsl = slice(nb * NT, (nb + 1) * NT)
                    ho = tpool.tile([P, NT], BF16, tag="ho", name="ho")
                    nc.vector.tensor_mul(
                        ho[:, :], cf_sb[nb][:, :], gate_sb[("o", nb)][:, :]
                    )
                    nc.scalar.dma_start(hT_t[:, oc, bsl], ho[:, :])

            # ---- prologue: first copy's working set -----------------------
            nc.scalar.dma_start(b_sb[:, :, :], bA[:, :, :])
            load_z(0)
            load_w(0)
            load_w(1)
            load_c(0)
            load_c(1)

            scratch = (
                zpool.tile([P, 4, B_L], BF16, tag="scr", name="scr")
                if extra_dma else None
            )

            if loop_r:
                with tc.For_i(0, loop_r // NCOPIES, 1):
                    for u in range(NU):
                        # prefetch one quarter of copy c+1's z per unit —
                        # slot (c+1)%ZS was last read by copy c-1, already
                        # retired; all 4 quarters land across copy c
                        if whole_z:
                            if u % OC == 0:
                                load_z(u // OC + 1)
                        else:
                            load_z(u // OC + 1, quarter=u % OC)
                        load_w(u + 2)          # slot (u+2)%WS, wraps to next
                        load_c(u + 2)
                        for _ in range(extra_dma):
                            ksl = slice((u % OC) * 4, (u % OC) * 4 + 4)
                            nc.sync.dma_start(
                                scratch[:, :, :], zT_t[:, ksl, :]
                            )
                        if u == 1:
                            nc.scalar.dma_start(b_sb[:, :, :], bA[:, :, :])
                        compute_unit(u)
            else:
                # one-shot: single copy, stream w/c two units ahead
                for u in range(OC):
                    if u + 2 < OC:
                        load_w(u + 2)
                        load_c(u + 2)
                    compute_unit(u)

    nc.finalize()
    return nc


def _get_nc():
    if "nc" not in _NC_CACHE:
        _NC_CACHE["nc"] = _build_nc()
    return _NC_CACHE["nc"]


def _shard_inputs(x, h, c, w_f, b_f, w_i, b_i, w_c, b_c, w_o, b_o):
    ws = {"f": w_f, "i": w_i, "c": w_c, "o": w_o}
    bz = {"f": b_f, "i": b_i, "c": b_c, "o": b_o}
    f32 = np.float32

    # per-out-group fused weight/bias shards (shared by the 4 batch groups)
    wB_sh = {}
    wQ_sh = {}
    bA_sh = {}
    for j in range(RO):
        cols = slice(j * O_L, (j + 1) * O_L)

        def units(g):
            # [KO, kp, OC, p] f32 view of w_g's column shard
            return (
                np.asarray(ws[g][:, cols], dtype=f32)
                .reshape(KO, P, OC, P)
            )

        # bf16 units: c ko 0..15, then o ko MIX_O..15 scaled by 2^15 so its
        # PSUM partial sits on the same scale as the fp8 o partial
        wb = np.concatenate(
            [units("c"), units("o")[MIX_O:] * (SZ * SW)], axis=0
        )  # [BU, kp, OC, p]
        wB_sh[j] = np.ascontiguousarray(
            wb.transpose(2, 1, 0, 3).astype(NP_BF16)
        )
        # fp8 units: f ko 0..15, i ko 0..15, o ko 0..MIX_O-1 (x SW)
        wq = np.concatenate(
            [units("f"), units("i"), units("o")[:MIX_O]], axis=0
        )  # [QU, kp, OC, p]
        wQ_sh[j] = np.ascontiguousarray(
            np.clip(wq.transpose(2, 1, 0, 3) * SW, -240.0, 240.0)
            .astype(NP_FP8)
        )
        bA_sh[j] = np.ascontiguousarray(
            np.stack(
                [np.asarray(bz[g], dtype=f32).reshape(-1)[cols]
                 .reshape(OC, P).T
                 for g in GATES],
                axis=2,
            )
        )

    in_maps = []
    for i in range(RB):
        rows = slice(i * B_L, (i + 1) * B_L)
        zTf = np.concatenate(
            [np.asarray(x[rows], dtype=f32), np.asarray(h[rows], dtype=f32)],
            axis=1,
        ).T  # [K, B_L] f32
        zT = np.ascontiguousarray(zTf.astype(NP_BF16))
        zQv = np.ascontiguousarray(
            np.clip(zTf * SZ, -240.0, 240.0).astype(NP_FP8)
        )
        for j in range(RO):
            cT = np.ascontiguousarray(
                np.asarray(c[rows, j * O_L:(j + 1) * O_L], dtype=f32).T
                .astype(NP_BF16)
            )
            in_maps.append(
                {"zT": zT, "zQ": zQv, "cT": cT,
                 "wB": wB_sh[j], "wQ": wQ_sh[j], "bA": bA_sh[j]}
            )
    return in_maps


def _run(in_maps, trace=False, trace_cores=None):
    global last_exec_time_ns
    nc = _get_nc()
    res = run_bass_kernel_spmd(
        nc, in_maps, list(range(RB * RO)),
        trace=trace, trace_cores=trace_cores,
    )
    if trace:
        last_exec_time_ns = res.exec_time_ns
    return res.results


def kernel(x, h, c, w_f, b_f, w_i, b_i, w_c, b_c, w_o, b_o):
    in_maps = _shard_inputs(
        x, h, c, w_f, b_f, w_i, b_i, w_c, b_c, w_o, b_o
    )
    results = _run(in_maps)
    out = np.empty((B_FULL, OUT), np.float32)
    for i in range(RB):
        for j in range(RO):
            shard = results[i * RO + j]["hT"]  # [O_L, B_L] bf16
            out[i * B_L:(i + 1) * B_L, j * O_L:(j + 1) * O_L] = (
                shard.astype(np.float32).T
            )
    return out
